# revision 2
# baseline (speedup 1.0000x reference)
"""Trainium2 Bass kernel for nn_AttModule_mamba_cross (B=4,D=256,L=2048,E=512,N=16,K=7,R=16).

Sharding: 8 cores = 2 mamba blocks x 4 batches, one (block, batch) unit per core.
All cores run one SPMD program; a per-core flag selects whether the conv_ff(x)
path is blended in (self-block cores) or the raw input is used (cross-block
cores). Host sums each core pair's partial outputs.

V-engine-optimized rewrite: the DVE is the bottleneck (64 scans + z/yp muls),
so everything else is pushed to Scalar (Silu/Softplus/Exp activations) and
Tensor (identity-matmul accumulation), fp16 throughout, and the gate/silu
work is hoisted out of the scan loop so Scalar runs Exp-only there.
padding_mask is all-ones per the problem spec (fill: ones); it is applied
only at the final output store.
"""
import numpy as np

import concourse.bass as bass
import concourse.bacc as bacc
import concourse.mybir as mybir
import concourse.tile as tile
from concourse import masks
from concourse.bass_utils import run_bass_kernel_spmd

B, D, L = 4, 256, 2048
E, N, K = 512, 16, 7
R = 16
EPS = 1e-5

F32 = mybir.dt.float32
FP16 = mybir.dt.float16
MULT = mybir.AluOpType.mult
ADD = mybir.AluOpType.add
SUB = mybir.AluOpType.subtract
AF = mybir.ActivationFunctionType

DT = D // 128   # 2 d-tiles
ET = E // 128   # 4 e-tiles
TC = L // 512   # 4 t-chunks of 512


def build_nc():
    nc = bacc.Bacc("TRN2", target_bir_lowering=False, debug=False, num_devices=8)

    # ---- DRAM I/O ----
    xin_d = nc.dram_tensor("xin", [D, L], F32, kind="ExternalInput")
    maskb_d = nc.dram_tensor("maskb", [128, L], FP16, kind="ExternalInput")
    flagv_d = nc.dram_tensor("flagv", [128, 1], F32, kind="ExternalInput")
    flag2_d = nc.dram_tensor("flag2", [128, 1], F32, kind="ExternalInput")
    ffw_d = nc.dram_tensor("ffw", [K, D, D], FP16, kind="ExternalInput")
    ffb_d = nc.dram_tensor("ffb", [D, 1], F32, kind="ExternalInput")
    w_inT_d = nc.dram_tensor("w_inT", [D, 2 * E], FP16, kind="ExternalInput")
    cw_d = nc.dram_tensor("cw", [E, K], F32, kind="ExternalInput")
    cb_d = nc.dram_tensor("cb", [E, 1], F32, kind="ExternalInput")
    w_xT_d = nc.dram_tensor("w_xT", [E, R + 2 * N], FP16, kind="ExternalInput")
    w_dtT_d = nc.dram_tensor("w_dtT", [R, E], FP16, kind="ExternalInput")
    dtb_d = nc.dram_tensor("dtb", [E, 1], F32, kind="ExternalInput")
    aneg_d = nc.dram_tensor("aneg", [E, N], F32, kind="ExternalInput")
    dp_d = nc.dram_tensor("dp", [E, 1], F32, kind="ExternalInput")
    w_outT_d = nc.dram_tensor("w_outT", [E, D], FP16, kind="ExternalInput")
    out_d = nc.dram_tensor("out", [D, L], F32, kind="ExternalOutput")

    with tile.TileContext(nc) as tc:
        _emit(nc, tc, locals())
    nc.compile()
    return nc


def _emit(nc, tc, d):
    xin_d, maskb_d, flagv_d, flag2_d = d["xin_d"], d["maskb_d"], d["flagv_d"], d["flag2_d"]
    ffw_d, ffb_d, w_inT_d = d["ffw_d"], d["ffb_d"], d["w_inT_d"]
    cw_d, cb_d, w_xT_d, w_dtT_d = d["cw_d"], d["cb_d"], d["w_xT_d"], d["w_dtT_d"]
    dtb_d, aneg_d, dp_d, w_outT_d, out_d = d["dtb_d"], d["aneg_d"], d["dp_d"], d["w_outT_d"], d["out_d"]

    _pools = []

    def pool(name, bufs, space="SBUF"):
        p = tc.alloc_tile_pool(name=name, bufs=bufs, space=space)
        _pools.append(p)
        return p

    wpool = pool("weights", 1)          # persistent small weights/constants
    big = pool("big", 1)                # persistent big activations
    chunk = pool("chunk512", 2)         # transient [128,512] tiles
    psmall = pool("psum_mm", 2, space="PSUM")    # [128,512] matmul tiles
    mmwp = pool("psum_mmw", 2, space="PSUM")     # [128,512] prelude matmul tiles
    dramp = pool("dram", 1, space="DRAM")
    # stage pools, created in reverse order of release (stack allocator is LIFO)
    stage2 = pool("stage2", 1)          # xp halo + diag_cw (released after dwconv)
    stage3 = pool("stage3", 1)          # mamba_in (released after gate)
    wff = pool("wff", 1)                # conv_ff weights + xin16 + convout (released after norm)
    stage1 = pool("stage1", 1)          # xin f32 (released after cast)

    # ---------------- stage1: input load + fp16 cast (issued first so the
    # conv_ff critical path starts as early as possible) ----------------
    xin = [stage1.tile([128, L], F32, tag=f"xinf{i}", name=f"xinf{i}") for i in range(DT)]
    for i in range(DT):
        nc.sync.dma_start(xin[i][:], xin_d[i * 128:(i + 1) * 128, :])
    ffw = [wff.tile([128, K * D], FP16, tag=f"ffw{i}", name=f"ffw{i}") for i in range(DT)]
    for i in range(DT):
        nc.sync.dma_start(
            ffw[i][:].rearrange("p (k d) -> p k d", k=K),
            ffw_d[:, i * 128:(i + 1) * 128, :].rearrange("k p d -> p k d"),
        )
    # padded fp16 input (3 zeros each side for the K=7 same-padding conv)
    xin16 = [wff.tile([128, L + 6], FP16, tag=f"xin16_{i}", name=f"xin16_{i}") for i in range(DT)]
    for i in range(DT):
        nc.gpsimd.memset(xin16[i][:, 0:3], 0.0)
        nc.gpsimd.memset(xin16[i][:, L + 3:], 0.0)
        nc.vector.tensor_copy(xin16[i][:, 3:L + 3], xin[i][:])
    stage1.release()

    # ---------------- persistent weights ----------------
    maskb = wpool.tile([128, L], FP16, tag="maskb", name="maskb")
    nc.sync.dma_start(maskb[:], maskb_d[:])
    flagv = wpool.tile([128, 1], F32, tag="flagv", name="flagv")
    flag2 = wpool.tile([128, 1], F32, tag="flag2", name="flag2")
    nc.sync.dma_start(flagv[:], flagv_d[:])
    nc.sync.dma_start(flag2[:], flag2_d[:])
    ffb = [wpool.tile([128, 1], F32, tag=f"ffb{i}", name=f"ffb{i}") for i in range(DT)]
    for i in range(DT):
        nc.sync.dma_start(ffb[i][:], ffb_d[i * 128:(i + 1) * 128, :])
    w_inT = [wpool.tile([128, 2 * E], FP16, tag=f"w_inT{i}", name=f"w_inT{i}") for i in range(DT)]
    for i in range(DT):
        nc.sync.dma_start(w_inT[i][:], w_inT_d[i * 128:(i + 1) * 128, :])
    cw = [wpool.tile([128, K], F32, tag=f"cw{i}", name=f"cw{i}") for i in range(ET)]
    cb = [wpool.tile([128, 1], F32, tag=f"cb{i}", name=f"cb{i}") for i in range(ET)]
    dtb = [wpool.tile([128, 1], F32, tag=f"dtb{i}", name=f"dtb{i}") for i in range(ET)]
    dp = [wpool.tile([128, 1], F32, tag=f"dp{i}", name=f"dp{i}") for i in range(ET)]
    aneg = [wpool.tile([128, N], F32, tag=f"aneg{i}", name=f"aneg{i}") for i in range(ET)]
    w_xT = [wpool.tile([128, R + 2 * N], FP16, tag=f"w_xT{i}", name=f"w_xT{i}") for i in range(ET)]
    w_outT = [wpool.tile([128, D], FP16, tag=f"w_outT{i}", name=f"w_outT{i}") for i in range(ET)]
    for i in range(ET):
        sl = slice(i * 128, (i + 1) * 128)
        nc.sync.dma_start(cw[i][:], cw_d[sl, :])
        nc.sync.dma_start(cb[i][:], cb_d[sl, :])
        nc.sync.dma_start(dtb[i][:], dtb_d[sl, :])
        nc.sync.dma_start(dp[i][:], dp_d[sl, :])
        nc.sync.dma_start(aneg[i][:], aneg_d[sl, :])
        nc.sync.dma_start(w_xT[i][:], w_xT_d[sl, :])
        nc.sync.dma_start(w_outT[i][:], w_outT_d[sl, :])
    w_dtT = wpool.tile([R, E], FP16, tag="w_dtT", name="w_dtT")
    nc.sync.dma_start(w_dtT[:], w_dtT_d[:])

    ident = wpool.tile([128, 128], F32, tag="ident", name="ident")
    masks.make_identity(nc, ident[:])
    identb = wpool.tile([128, 128], FP16, tag="identb", name="identb")
    nc.vector.tensor_copy(identb[:], ident[:])
    # flag-scaled identity: adds resid into the out_proj psum on self cores only
    flagident = wpool.tile([128, 128], FP16, tag="flagident", name="flagident")
    nc.vector.tensor_scalar_mul(flagident[:], ident[:], flagv[:])
    diag_dp = [wpool.tile([128, 128], FP16, tag=f"ddp{e}", name=f"ddp{e}") for e in range(ET)]
    for e in range(ET):
        nc.vector.tensor_scalar_mul(diag_dp[e][:], ident[:], dp[e][:])
    diag_cw = [[stage2.tile([128, 128], FP16, tag=f"dcw{e}_{k}", name=f"dcw{e}_{k}") for k in range(K)]
               for e in range(ET)]
    for e in range(ET):
        for k in range(K):
            nc.vector.tensor_scalar_mul(diag_cw[e][k][:], ident[:], cw[e][:, k:k + 1])

    # ---------------- conv_ff -> convout (fp16) + resid (fp16) ----------------
    convout = [wff.tile([128, L], FP16, tag=f"convout{i}", name=f"convout{i}") for i in range(DT)]
    resid_dram = dramp.tile([D, L], FP16, tag="residd", name="residd")
    for do in range(DT):
        for t in range(TC):
            ps = mmwp.tile([128, 512], F32, tag="mmw", name="psmmw")
            nmm = K * DT
            i = 0
            for k in range(K):
                for di in range(DT):
                    nc.tensor.matmul(
                        ps[:],
                        ffw[di][:, k * D + do * 128: k * D + (do + 1) * 128],
                        xin16[di][:, t * 512 + k: t * 512 + k + 512],
                        start=(i == 0), stop=(i == nmm - 1),
                    )
                    i += 1
            nc.scalar.activation(convout[do][:, t * 512:(t + 1) * 512], ps[:], AF.Relu, bias=ffb[do][:])
            resid_s = chunk.tile([128, 512], FP16, tag="resid_s", name="resid_s")
            nc.vector.tensor_tensor(
                resid_s[:],
                convout[do][:, t * 512:(t + 1) * 512],
                xin16[do][:, 3 + t * 512: 3 + (t + 1) * 512], ADD,
            )
            nc.sync.dma_start(
                resid_dram[do * 128:(do + 1) * 128, t * 512:(t + 1) * 512], resid_s[:])

    # ---------------- instance norm -> mamba_in (fp16) ----------------
    # norm input: self cores = convout, cross cores = xin  (flagv selects)
    mamba_in = [stage3.tile([128, L], FP16, tag=f"mambain{i}", name=f"mambain{i}") for i in range(DT)]
    for i in range(DT):
        nin = chunk.tile([128, L], FP16, tag="nin2048", name="nin")
        nc.vector.tensor_scalar_mul(nin[:], xin16[i][:, 3:L + 3], flag2[:])
        nc.vector.scalar_tensor_tensor(nin[:], convout[i][:], flagv[:], nin[:], MULT, ADD)
        ssum = wpool.tile([128, 1], F32, tag=f"ssum{i}", name=f"ssum{i}")
        ssq = wpool.tile([128, 1], F32, tag=f"ssq{i}", name=f"ssq{i}")
        trash = chunk.tile([128, L], FP16, tag="nin2048", name="trash")
        nc.vector.tensor_reduce(ssum[:], nin[:], mybir.AxisListType.X, ADD)
        nc.scalar.activation(trash[:], nin[:], AF.Square, accum_out=ssq[:])
        mean = wpool.tile([128, 1], F32, tag=f"mean{i}", name=f"mean{i}")
        var = wpool.tile([128, 1], F32, tag=f"var{i}", name=f"var{i}")
        nc.vector.tensor_scalar_mul(mean[:], ssum[:], 1.0 / L)
        nc.vector.tensor_scalar_mul(var[:], ssq[:], 1.0 / L)
        msq = wpool.tile([128, 1], F32, tag=f"msq{i}", name=f"msq{i}")
        nc.vector.tensor_tensor(msq[:], mean[:], mean[:], MULT)
        nc.vector.scalar_tensor_tensor(var[:], msq[:], -1.0, var[:], MULT, ADD)
        nc.vector.tensor_scalar_add(var[:], var[:], EPS)
        inv = wpool.tile([128, 1], F32, tag=f"inv{i}", name=f"inv{i}")
        nc.vector.reciprocal(inv[:], var[:])
        nc.scalar.sqrt(inv[:], inv[:])
        muinv = wpool.tile([128, 1], F32, tag=f"muinv{i}", name=f"muinv{i}")
        nc.vector.tensor_tensor(muinv[:], mean[:], inv[:], MULT)
        nc.vector.tensor_scalar(mamba_in[i][:], nin[:], inv[:], muinv[:], MULT, SUB)
    wff.release()

    # ---------------- in_proj (xp half) ----------------
    xp = [stage2.tile([128, L + 6], FP16, tag=f"xp{e}", name=f"xp{e}") for e in range(ET)]
    for e in range(ET):
        nc.gpsimd.memset(xp[e][:, 0:6], 0.0)
        for t in range(TC):
            ps = mmwp.tile([128, 512], F32, tag="mmw", name="psmmw")
            for di in range(DT):
                nc.tensor.matmul(
                    ps[:], w_inT[di][:, e * 128:(e + 1) * 128],
                    mamba_in[di][:, t * 512:(t + 1) * 512],
                    start=(di == 0), stop=(di == DT - 1),
                )
            nc.vector.tensor_copy(xp[e][:, 6 + t * 512: 6 + (t + 1) * 512], ps[:])

    # ---------------- gate half: zs = silu(in_proj_z), spilled to DRAM ----------------
    zs_dram = dramp.tile([E, L], FP16, tag="zsd", name="zsd")
    for e in range(ET):
        for t in range(TC):
            ps = psmall.tile([128, 512], F32, tag="mm", name="psmm")
            for di in range(DT):
                nc.tensor.matmul(
                    ps[:], w_inT[di][:, E + e * 128: E + (e + 1) * 128],
                    mamba_in[di][:, t * 512:(t + 1) * 512],
                    start=(di == 0), stop=(di == DT - 1),
                )
            zs_s = chunk.tile([128, 512], FP16, tag="zs_s", name="zs_s")
            nc.scalar.activation(zs_s[:], ps[:], AF.Silu)
            nc.sync.dma_start(
                zs_dram[e * 128:(e + 1) * 128, t * 512:(t + 1) * 512], zs_s[:])
    stage3.release()

    # ---------------- depthwise causal conv + silu -> u ----------------
    u = [big.tile([128, L], FP16, tag=f"u{e}", name=f"u{e}") for e in range(ET)]
    for e in range(ET):
        for t in range(TC):
            ps = mmwp.tile([128, 512], F32, tag="mmw", name="psmmw")
            for k in range(K):
                nc.tensor.matmul(
                    ps[:], diag_cw[e][k][:],
                    xp[e][:, t * 512 + k: t * 512 + k + 512],
                    start=(k == 0), stop=(k == K - 1),
                )
            nc.scalar.activation(u[e][:, t * 512:(t + 1) * 512], ps[:], AF.Silu, bias=cb[e][:])
    stage2.release()

    # ---------------- x_proj -> xdblR (fp16) + B/C rows to DRAM ----------------
    xdblR = big.tile([R, L], FP16, tag="xdblR", name="xdblR")
    bcsrc = big.tile([2 * N, L], FP16, tag="bcsrc", name="bcsrc")
    for t in range(TC):
        ps = mmwp.tile([R + 2 * N, 512], F32, tag="mmw", name="psmmx")
        for e in range(ET):
            nc.tensor.matmul(
                ps[:], w_xT[e][:], u[e][:, t * 512:(t + 1) * 512],
                start=(e == 0), stop=(e == ET - 1),
            )
        nc.scalar.copy(bcsrc[:, t * 512:(t + 1) * 512], ps[0:2 * N, :])
        nc.scalar.copy(xdblR[:, t * 512:(t + 1) * 512], ps[2 * N:2 * N + R, :])
    bc_dram = dramp.tile([2 * N, L], FP16, tag="bcdram", name="bcdram")
    for t in range(TC):
        nc.sync.dma_start(bc_dram[:, t * 512:(t + 1) * 512],
                          bcsrc[:, t * 512:(t + 1) * 512])

    bcp = pool("bcast", 4)
    scanp = pool("scan", 2)

    # ---------------- dt_proj -> dt = -softplus(...) via ln(sigmoid(-x)) ----------------
    # (no Softplus table on gen3; sigmoid/ln batched over e-tile PAIRS to
    # halve the Act table reloads)
    dt = [big.tile([128, L], FP16, tag=f"dt{e}", name=f"dt{e}") for e in range(ET)]
    for ep in range(ET // 2):
        lntmps = {}
        for e in (2 * ep, 2 * ep + 1):
            lntmp = chunk.tile([128, L], F32, tag="lntmp", name=f"lntmp{e}")
            lntmps[e] = lntmp
            for t in range(TC):
                ps = psmall.tile([128, 512], F32, tag="mm", name="psmm")
                nc.tensor.matmul(
                    ps[:], w_dtT[:, e * 128:(e + 1) * 128],
                    xdblR[:, t * 512:(t + 1) * 512], start=True, stop=True,
                )
                nc.scalar.activation(lntmp[:, t * 512:(t + 1) * 512], ps[:], AF.Sigmoid,
                                     bias=dtb[e][:], scale=-1.0)
        for e in (2 * ep, 2 * ep + 1):
            for t in range(TC):
                nc.scalar.activation(dt[e][:, t * 512:(t + 1) * 512],
                                     lntmps[e][:, t * 512:(t + 1) * 512], AF.Ln)

    w = [big.tile([128, L], FP16, tag=f"w{e}", name=f"w{e}") for e in range(ET)]
    for e in range(ET):
        nc.vector.tensor_tensor(w[e][:], dt[e][:], u[e][:], MULT)

    # ---------------- selective scan ----------------
    # Concatenated scans: the tensor_tensor_scan instruction has a ~4us fixed
    # cost, so pack NCAT n-states into one long scan per (e, quad). State
    # resets at unit boundaries are free: h[0] = dA[0]*h[-1] + z[0] and the
    # boundary dA column is pinned to zero (Exp writes skip it; a per-set
    # memset clears it), so each unit starts from h=0 exactly.
    # n-outer over e-pairs: b/c broadcast once per (pass, n) instead of per
    # (e, n); PSUM holds two full-L f32 y-accumulators (8 banks).
    NCAT = 4
    LCAT = NCAT * L
    mmwp.release()
    psmall.release()
    pbig = pool("psum_y", 1, space="PSUM")       # 2x [128,2048] y accumulators
    yg = [None] * ET
    for pair in range(ET // 2):
        es = [2 * pair, 2 * pair + 1]
        py = {}
        for e in es:
            py[e] = pbig.tile([128, L], F32, tag=f"py{e - 2 * pair}", name=f"py{e}")
            for t in range(TC):
                nc.tensor.matmul(
                    py[e][:, t * 512:(t + 1) * 512], diag_dp[e][:],
                    u[e][:, t * 512:(t + 1) * 512],
                    start=True, stop=False,
                )
        for q in range(N // NCAT):
            ns = list(range(q * NCAT, (q + 1) * NCAT))
            bcs = []
            for n in ns:
                b_bc = bcp.tile([128, L], FP16, tag="bbc", name="bbc")
                c_bc = bcp.tile([128, L], FP16, tag="cbc", name="cbc")
                nc.sync.dma_start(b_bc[:], bc_dram[n, :].partition_broadcast(128))
                nc.sync.dma_start(c_bc[:], bc_dram[N + n, :].partition_broadcast(128))
                bcs.append((b_bc, c_bc))
            for e in es:
                dA = scanp.tile([128, LCAT], FP16, tag="dA", name="dA")
                z = scanp.tile([128, LCAT], FP16, tag="z", name="z")
                for k, n in enumerate(ns):
                    if k > 0:
                        nc.vector.memset(dA[:, k * L:k * L + 1], 0.0)
                    off = k * L + (1 if k > 0 else 0)
                    nc.scalar.activation(dA[:, off:(k + 1) * L],
                                         dt[e][:, off - k * L:L], AF.Exp,
                                         scale=aneg[e][:, n:n + 1])
                    # z slices all on DVE (Pool SBUF-port contention test)
                    nc.vector.tensor_tensor(z[:, k * L:(k + 1) * L], w[e][:],
                                            bcs[k][0][:], MULT)
                # in-place scan: h overwrites z (same-position write-behind)
                nc.vector.tensor_tensor_scan(z[:], dA[:], z[:], 0.0, MULT, ADD)
                # yp slices in-place over dA, then accumulate into py via PE
                for k, n in enumerate(ns):
                    nc.vector.tensor_tensor(dA[:, k * L:(k + 1) * L],
                                            z[:, k * L:(k + 1) * L],
                                            bcs[k][1][:], MULT)
                    for t in range(TC):
                        nc.tensor.matmul(
                            py[e][:, t * 512:(t + 1) * 512], identb[:],
                            dA[:, k * L + t * 512: k * L + (t + 1) * 512],
                            start=False, stop=(q == N // NCAT - 1 and k == NCAT - 1),
                        )
        # ---- gating: yg = py * zs; Act evacuates PSUM so the DVE mul runs
        # at the 2x fp16 rate on SBUF operands ----
        for e in es:
            yge = big.tile([128, L], FP16, tag=f"u{e}", name=f"yg{e}")
            yg[e] = yge
            for t in range(TC):
                zs_l = chunk.tile([128, 512], FP16, tag="zs_s", name="zs_l")
                nc.sync.dma_start(
                    zs_l[:], zs_dram[e * 128:(e + 1) * 128, t * 512:(t + 1) * 512])
                py_s = chunk.tile([128, 512], FP16, tag="py_s", name="py_s")
                nc.scalar.copy(py_s[:], py[e][:, t * 512:(t + 1) * 512])
                nc.vector.tensor_tensor(
                    yge[:, t * 512:(t + 1) * 512],
                    py_s[:],
                    zs_l[:], MULT,
                )
    pbig.release()
    psout = pool("psum_out", 2, space="PSUM")

    # ---------------- out_proj + resid-add (via matmul) + mask + store ----------------
    for do in range(DT):
        pss = [psout.tile([128, 512], F32, tag=f"op{t}", bufs=1, name=f"op{do}_{t}")
               for t in range(TC)]
        for e in range(ET):
            for t in range(TC):
                nc.tensor.matmul(
                    pss[t][:], w_outT[e][:, do * 128:(do + 1) * 128],
                    yg[e][:, t * 512:(t + 1) * 512],
                    start=(e == 0), stop=False,
                )
        for t in range(TC):
            resid_l = chunk.tile([128, 512], FP16, tag="resid_s", name="resid_l")
            nc.sync.dma_start(
                resid_l[:], resid_dram[do * 128:(do + 1) * 128, t * 512:(t + 1) * 512])
            nc.tensor.matmul(
                pss[t][:], flagident[:],
                resid_l[:],
                start=False, stop=True,
            )
            ofin = chunk.tile([128, 512], F32, tag="ofin", name="ofin")
            nc.vector.tensor_tensor(ofin[:], pss[t][:], maskb[:, t * 512:(t + 1) * 512], MULT)
            nc.sync.dma_start(out_d[do * 128:(do + 1) * 128, t * 512:(t + 1) * 512], ofin[:])

    for p in reversed(_pools):
        if not p._released:
            p.release()


_NC_CACHE = {}


def _get_nc():
    if "nc" not in _NC_CACHE:
        _NC_CACHE["nc"] = build_nc()
    return _NC_CACHE["nc"]


def _core_inputs(blk, b, inputs):
    pfx = "s_" if blk == 0 else "c_"
    xin = inputs["x"][b] if blk == 0 else inputs["encoder_states"][b]
    f = 1.0 if blk == 0 else 0.0
    g = lambda k: np.asarray(inputs[pfx + k])
    aneg = np.exp(g("A_log"))  # = -A; dt tile holds -softplus so dA = exp(aneg*dt)
    return {
        "xin": np.ascontiguousarray(xin, np.float32),
        "maskb": np.ascontiguousarray(
            np.broadcast_to(inputs["padding_mask"][b][None, :], (128, L))).astype(np.float16),
        "flagv": np.full((128, 1), f, np.float32),
        "flag2": np.full((128, 1), 1.0 - f, np.float32),
        "ffw": np.ascontiguousarray(np.asarray(inputs["ff_w"]).transpose(2, 1, 0)).astype(np.float16),
        "ffb": np.asarray(inputs["ff_b"]).reshape(D, 1).astype(np.float32),
        "w_inT": np.ascontiguousarray(g("in_proj_w").T).astype(np.float16),
        "cw": np.ascontiguousarray(g("conv_w").reshape(E, K), np.float32),
        "cb": g("conv_b").reshape(E, 1).astype(np.float32),
        "w_xT": np.ascontiguousarray(
            g("x_proj_w").T[:, list(range(R, R + 2 * N)) + list(range(R))]
        ).astype(np.float16),
        "w_dtT": np.ascontiguousarray(g("dt_proj_w").T).astype(np.float16),
        "dtb": (-g("dt_proj_b")).reshape(E, 1).astype(np.float32),
        "aneg": np.ascontiguousarray(aneg, np.float32),
        "dp": (-g("D")).reshape(E, 1).astype(np.float32),
        "w_outT": np.ascontiguousarray(-g("out_proj_w").T).astype(np.float16),
    }


def kernel(**inputs):
    nc = _get_nc()
    in_maps = []
    for b in range(B):
        in_maps.append(_core_inputs(0, b, inputs))  # core 2b: self block
        in_maps.append(_core_inputs(1, b, inputs))  # core 2b+1: cross block
    res = run_bass_kernel_spmd(nc, in_maps, core_ids=list(range(8)))
    out = np.empty((B, D, L), np.float32)
    for b in range(B):
        out[b] = res.results[2 * b]["out"] + res.results[2 * b + 1]["out"]
    return out



# revision 4
# speedup vs baseline: 1.0113x; 1.0113x over previous
"""Trainium2 Bass kernel for nn_AttModule_mamba_cross (B=4,D=256,L=2048,E=512,N=16,K=7,R=16).

Sharding: 8 cores = 2 mamba blocks x 4 batches, one (block, batch) unit per core.
All cores run one SPMD program; a per-core flag selects whether the conv_ff(x)
path is blended in (self-block cores) or the raw input is used (cross-block
cores). Host sums each core pair's partial outputs.

DVE-centric design (the scan engine is the bottleneck; measured costs drove
every choice):
- The selective scan runs as 16 concatenated tensor_tensor_scan instructions
  ([128, 4*2048]): 4 n-states share one scan, with the boundary dA column
  pinned to 0 so the recurrence state resets exactly between units.
- All elementwise muls (z = w*b, yp = h*c) stay on the DVE: GpSimd shares
  SBUF ports with the DVE and any concurrent Pool work slows DVE tensor ops
  far more than it helps.
- n-outer over e-tile pairs: B/C rows are broadcast once per (pass, n) via
  DMA instead of once per (e, n); PSUM holds two full-L f32 y-accumulators.
- resid/zs spill to DRAM to make room for the wide scan tiles; gating
  evacuates PSUM through the Act engine so the DVE mul runs at the 2x fp16
  rate; sigmoid/ln run batched per e-pair to halve Act table reloads.
padding_mask is all-ones per the problem spec (fill: ones); it is applied
only at the final output store.
"""
import numpy as np

import concourse.bass as bass
import concourse.bacc as bacc
import concourse.mybir as mybir
import concourse.tile as tile
from concourse import masks
from concourse.bass_utils import run_bass_kernel_spmd

B, D, L = 4, 256, 2048
E, N, K = 512, 16, 7
R = 16
EPS = 1e-5

F32 = mybir.dt.float32
FP16 = mybir.dt.float16
MULT = mybir.AluOpType.mult
ADD = mybir.AluOpType.add
SUB = mybir.AluOpType.subtract
AF = mybir.ActivationFunctionType

DT = D // 128   # 2 d-tiles
ET = E // 128   # 4 e-tiles
TC = L // 512   # 4 t-chunks of 512


def build_nc():
    nc = bacc.Bacc("TRN2", target_bir_lowering=False, debug=False, num_devices=8)

    # ---- DRAM I/O ----
    xin_d = nc.dram_tensor("xin", [D, L], F32, kind="ExternalInput")
    maskb_d = nc.dram_tensor("maskb", [128, L], FP16, kind="ExternalInput")
    flagv_d = nc.dram_tensor("flagv", [128, 1], F32, kind="ExternalInput")
    flag2_d = nc.dram_tensor("flag2", [128, 1], F32, kind="ExternalInput")
    ffw_d = nc.dram_tensor("ffw", [K, D, D], FP16, kind="ExternalInput")
    ffb_d = nc.dram_tensor("ffb", [D, 1], F32, kind="ExternalInput")
    w_inT_d = nc.dram_tensor("w_inT", [D, 2 * E], FP16, kind="ExternalInput")
    cw_d = nc.dram_tensor("cw", [E, K], F32, kind="ExternalInput")
    cb_d = nc.dram_tensor("cb", [E, 1], F32, kind="ExternalInput")
    w_xT_d = nc.dram_tensor("w_xT", [E, R + 2 * N], FP16, kind="ExternalInput")
    w_dtT_d = nc.dram_tensor("w_dtT", [R, E], FP16, kind="ExternalInput")
    dtb_d = nc.dram_tensor("dtb", [E, 1], F32, kind="ExternalInput")
    aneg_d = nc.dram_tensor("aneg", [E, N], F32, kind="ExternalInput")
    dp_d = nc.dram_tensor("dp", [E, 1], F32, kind="ExternalInput")
    w_outT_d = nc.dram_tensor("w_outT", [E, D], FP16, kind="ExternalInput")
    out_d = nc.dram_tensor("out", [D, L], F32, kind="ExternalOutput")

    with tile.TileContext(nc) as tc:
        _emit(nc, tc, locals())
    nc.compile()
    return nc


def _emit(nc, tc, d):
    xin_d, maskb_d, flagv_d, flag2_d = d["xin_d"], d["maskb_d"], d["flagv_d"], d["flag2_d"]
    ffw_d, ffb_d, w_inT_d = d["ffw_d"], d["ffb_d"], d["w_inT_d"]
    cw_d, cb_d, w_xT_d, w_dtT_d = d["cw_d"], d["cb_d"], d["w_xT_d"], d["w_dtT_d"]
    dtb_d, aneg_d, dp_d, w_outT_d, out_d = d["dtb_d"], d["aneg_d"], d["dp_d"], d["w_outT_d"], d["out_d"]

    _pools = []

    def pool(name, bufs, space="SBUF"):
        p = tc.alloc_tile_pool(name=name, bufs=bufs, space=space)
        _pools.append(p)
        return p

    wpool = pool("weights", 1)          # persistent small weights/constants
    big = pool("big", 1)                # persistent big activations
    chunk = pool("chunk512", 2)         # transient [128,512] tiles
    psmall = pool("psum_mm", 2, space="PSUM")    # [128,512] matmul tiles
    mmwp = pool("psum_mmw", 2, space="PSUM")     # [128,512] prelude matmul tiles
    dramp = pool("dram", 1, space="DRAM")
    # stage pools, created in reverse order of release (stack allocator is LIFO)
    stage2 = pool("stage2", 1)          # xp halo + diag_cw (released after dwconv)
    stage3 = pool("stage3", 1)          # mamba_in (released after gate)
    wff = pool("wff", 1)                # conv_ff weights + xin16 + convout (released after norm)
    stage1 = pool("stage1", 1)          # xin f32 (released after cast)

    # ---------------- stage1: input load + fp16 cast (issued first so the
    # conv_ff critical path starts as early as possible) ----------------
    xin = [stage1.tile([128, L], F32, tag=f"xinf{i}", name=f"xinf{i}") for i in range(DT)]
    for i in range(DT):
        nc.sync.dma_start(xin[i][:], xin_d[i * 128:(i + 1) * 128, :])
    ffw = [wff.tile([128, K * D], FP16, tag=f"ffw{i}", name=f"ffw{i}") for i in range(DT)]
    for i in range(DT):
        nc.sync.dma_start(
            ffw[i][:].rearrange("p (k d) -> p k d", k=K),
            ffw_d[:, i * 128:(i + 1) * 128, :].rearrange("k p d -> p k d"),
        )
    # padded fp16 input (3 zeros each side for the K=7 same-padding conv)
    xin16 = [wff.tile([128, L + 6], FP16, tag=f"xin16_{i}", name=f"xin16_{i}") for i in range(DT)]
    for i in range(DT):
        nc.gpsimd.memset(xin16[i][:, 0:3], 0.0)
        nc.gpsimd.memset(xin16[i][:, L + 3:], 0.0)
        nc.vector.tensor_copy(xin16[i][:, 3:L + 3], xin[i][:])
    stage1.release()

    # ---------------- persistent weights ----------------
    maskb = wpool.tile([128, L], FP16, tag="maskb", name="maskb")
    nc.sync.dma_start(maskb[:], maskb_d[:])
    flagv = wpool.tile([128, 1], F32, tag="flagv", name="flagv")
    flag2 = wpool.tile([128, 1], F32, tag="flag2", name="flag2")
    nc.sync.dma_start(flagv[:], flagv_d[:])
    nc.sync.dma_start(flag2[:], flag2_d[:])
    ffb = [wpool.tile([128, 1], F32, tag=f"ffb{i}", name=f"ffb{i}") for i in range(DT)]
    for i in range(DT):
        nc.sync.dma_start(ffb[i][:], ffb_d[i * 128:(i + 1) * 128, :])
    w_inT = [wpool.tile([128, 2 * E], FP16, tag=f"w_inT{i}", name=f"w_inT{i}") for i in range(DT)]
    for i in range(DT):
        nc.sync.dma_start(w_inT[i][:], w_inT_d[i * 128:(i + 1) * 128, :])
    cw = [wpool.tile([128, K], F32, tag=f"cw{i}", name=f"cw{i}") for i in range(ET)]
    cb = [wpool.tile([128, 1], F32, tag=f"cb{i}", name=f"cb{i}") for i in range(ET)]
    dtb = [wpool.tile([128, 1], F32, tag=f"dtb{i}", name=f"dtb{i}") for i in range(ET)]
    dp = [wpool.tile([128, 1], F32, tag=f"dp{i}", name=f"dp{i}") for i in range(ET)]
    aneg = [wpool.tile([128, N], F32, tag=f"aneg{i}", name=f"aneg{i}") for i in range(ET)]
    w_xT = [wpool.tile([128, R + 2 * N], FP16, tag=f"w_xT{i}", name=f"w_xT{i}") for i in range(ET)]
    w_outT = [wpool.tile([128, D], FP16, tag=f"w_outT{i}", name=f"w_outT{i}") for i in range(ET)]
    for i in range(ET):
        sl = slice(i * 128, (i + 1) * 128)
        nc.sync.dma_start(cw[i][:], cw_d[sl, :])
        nc.sync.dma_start(cb[i][:], cb_d[sl, :])
        nc.sync.dma_start(dtb[i][:], dtb_d[sl, :])
        nc.sync.dma_start(dp[i][:], dp_d[sl, :])
        nc.sync.dma_start(aneg[i][:], aneg_d[sl, :])
        nc.sync.dma_start(w_xT[i][:], w_xT_d[sl, :])
        nc.sync.dma_start(w_outT[i][:], w_outT_d[sl, :])
    w_dtT = wpool.tile([R, E], FP16, tag="w_dtT", name="w_dtT")
    nc.sync.dma_start(w_dtT[:], w_dtT_d[:])

    ident = wpool.tile([128, 128], F32, tag="ident", name="ident")
    masks.make_identity(nc, ident[:])
    identb = wpool.tile([128, 128], FP16, tag="identb", name="identb")
    nc.vector.tensor_copy(identb[:], ident[:])
    # flag-scaled identity: adds resid into the out_proj psum on self cores only
    flagident = wpool.tile([128, 128], FP16, tag="flagident", name="flagident")
    nc.vector.tensor_scalar_mul(flagident[:], ident[:], flagv[:])
    diag_dp = [wpool.tile([128, 128], FP16, tag=f"ddp{e}", name=f"ddp{e}") for e in range(ET)]
    for e in range(ET):
        nc.vector.tensor_scalar_mul(diag_dp[e][:], ident[:], dp[e][:])
    diag_cw = [[stage2.tile([128, 128], FP16, tag=f"dcw{e}_{k}", name=f"dcw{e}_{k}") for k in range(K)]
               for e in range(ET)]
    for e in range(ET):
        for k in range(K):
            nc.vector.tensor_scalar_mul(diag_cw[e][k][:], ident[:], cw[e][:, k:k + 1])

    # ---------------- conv_ff -> convout (fp16) + resid (fp16) ----------------
    convout = [wff.tile([128, L], FP16, tag=f"convout{i}", name=f"convout{i}") for i in range(DT)]
    resid_dram = dramp.tile([D, L], FP16, tag="residd", name="residd")
    for do in range(DT):
        for t in range(TC):
            ps = mmwp.tile([128, 512], F32, tag="mmw", name="psmmw")
            nmm = K * DT
            i = 0
            for k in range(K):
                for di in range(DT):
                    nc.tensor.matmul(
                        ps[:],
                        ffw[di][:, k * D + do * 128: k * D + (do + 1) * 128],
                        xin16[di][:, t * 512 + k: t * 512 + k + 512],
                        start=(i == 0), stop=(i == nmm - 1),
                    )
                    i += 1
            nc.scalar.activation(convout[do][:, t * 512:(t + 1) * 512], ps[:], AF.Relu, bias=ffb[do][:])
            resid_s = chunk.tile([128, 512], FP16, tag="resid_s", name="resid_s")
            nc.vector.tensor_tensor(
                resid_s[:],
                convout[do][:, t * 512:(t + 1) * 512],
                xin16[do][:, 3 + t * 512: 3 + (t + 1) * 512], ADD,
            )
            nc.sync.dma_start(
                resid_dram[do * 128:(do + 1) * 128, t * 512:(t + 1) * 512], resid_s[:])

    # ---------------- instance norm -> mamba_in (fp16) ----------------
    # norm input: self cores = convout, cross cores = xin  (flagv selects)
    mamba_in = [stage3.tile([128, L], FP16, tag=f"mambain{i}", name=f"mambain{i}") for i in range(DT)]
    for i in range(DT):
        nin = chunk.tile([128, L], FP16, tag="nin2048", name="nin")
        nc.vector.tensor_scalar_mul(nin[:], xin16[i][:, 3:L + 3], flag2[:])
        nc.vector.scalar_tensor_tensor(nin[:], convout[i][:], flagv[:], nin[:], MULT, ADD)
        ssum = wpool.tile([128, 1], F32, tag=f"ssum{i}", name=f"ssum{i}")
        ssq = wpool.tile([128, 1], F32, tag=f"ssq{i}", name=f"ssq{i}")
        trash = chunk.tile([128, L], FP16, tag="nin2048", name="trash")
        nc.vector.tensor_reduce(ssum[:], nin[:], mybir.AxisListType.X, ADD)
        nc.scalar.activation(trash[:], nin[:], AF.Square, accum_out=ssq[:])
        mean = wpool.tile([128, 1], F32, tag=f"mean{i}", name=f"mean{i}")
        var = wpool.tile([128, 1], F32, tag=f"var{i}", name=f"var{i}")
        nc.vector.tensor_scalar_mul(mean[:], ssum[:], 1.0 / L)
        nc.vector.tensor_scalar_mul(var[:], ssq[:], 1.0 / L)
        msq = wpool.tile([128, 1], F32, tag=f"msq{i}", name=f"msq{i}")
        nc.vector.tensor_tensor(msq[:], mean[:], mean[:], MULT)
        nc.vector.scalar_tensor_tensor(var[:], msq[:], -1.0, var[:], MULT, ADD)
        nc.vector.tensor_scalar_add(var[:], var[:], EPS)
        inv = wpool.tile([128, 1], F32, tag=f"inv{i}", name=f"inv{i}")
        nc.vector.reciprocal(inv[:], var[:])
        nc.scalar.sqrt(inv[:], inv[:])
        muinv = wpool.tile([128, 1], F32, tag=f"muinv{i}", name=f"muinv{i}")
        nc.vector.tensor_tensor(muinv[:], mean[:], inv[:], MULT)
        nc.vector.tensor_scalar(mamba_in[i][:], nin[:], inv[:], muinv[:], MULT, SUB)
    wff.release()

    # ---------------- in_proj (xp half) ----------------
    xp = [stage2.tile([128, L + 6], FP16, tag=f"xp{e}", name=f"xp{e}") for e in range(ET)]
    for e in range(ET):
        nc.gpsimd.memset(xp[e][:, 0:6], 0.0)
        for t in range(TC):
            ps = mmwp.tile([128, 512], F32, tag="mmw", name="psmmw")
            for di in range(DT):
                nc.tensor.matmul(
                    ps[:], w_inT[di][:, e * 128:(e + 1) * 128],
                    mamba_in[di][:, t * 512:(t + 1) * 512],
                    start=(di == 0), stop=(di == DT - 1),
                )
            nc.vector.tensor_copy(xp[e][:, 6 + t * 512: 6 + (t + 1) * 512], ps[:])

    # ---------------- gate half: zs = silu(in_proj_z), spilled to DRAM ----------------
    zs_dram = dramp.tile([E, L], FP16, tag="zsd", name="zsd")
    for e in range(ET):
        for t in range(TC):
            ps = psmall.tile([128, 512], F32, tag="mm", name="psmm")
            for di in range(DT):
                nc.tensor.matmul(
                    ps[:], w_inT[di][:, E + e * 128: E + (e + 1) * 128],
                    mamba_in[di][:, t * 512:(t + 1) * 512],
                    start=(di == 0), stop=(di == DT - 1),
                )
            zs_s = chunk.tile([128, 512], FP16, tag="zs_s", name="zs_s")
            nc.scalar.activation(zs_s[:], ps[:], AF.Silu)
            nc.sync.dma_start(
                zs_dram[e * 128:(e + 1) * 128, t * 512:(t + 1) * 512], zs_s[:])
    stage3.release()

    # ---------------- depthwise causal conv + silu -> u ----------------
    u = [big.tile([128, L], FP16, tag=f"u{e}", name=f"u{e}") for e in range(ET)]
    for e in range(ET):
        for t in range(TC):
            ps = mmwp.tile([128, 512], F32, tag="mmw", name="psmmw")
            for k in range(K):
                nc.tensor.matmul(
                    ps[:], diag_cw[e][k][:],
                    xp[e][:, t * 512 + k: t * 512 + k + 512],
                    start=(k == 0), stop=(k == K - 1),
                )
            nc.scalar.activation(u[e][:, t * 512:(t + 1) * 512], ps[:], AF.Silu, bias=cb[e][:])
    stage2.release()

    # ---------------- x_proj -> xdblR (fp16) + B/C rows to DRAM ----------------
    xdblR = big.tile([R, L], FP16, tag="xdblR", name="xdblR")
    bcsrc = big.tile([2 * N, L], FP16, tag="bcsrc", name="bcsrc")
    for t in range(TC):
        ps = mmwp.tile([R + 2 * N, 512], F32, tag="mmw", name="psmmx")
        for e in range(ET):
            nc.tensor.matmul(
                ps[:], w_xT[e][:], u[e][:, t * 512:(t + 1) * 512],
                start=(e == 0), stop=(e == ET - 1),
            )
        nc.scalar.copy(bcsrc[:, t * 512:(t + 1) * 512], ps[0:2 * N, :])
        nc.scalar.copy(xdblR[:, t * 512:(t + 1) * 512], ps[2 * N:2 * N + R, :])
    bc_dram = dramp.tile([2 * N, L], FP16, tag="bcdram", name="bcdram")
    for t in range(TC):
        nc.sync.dma_start(bc_dram[:, t * 512:(t + 1) * 512],
                          bcsrc[:, t * 512:(t + 1) * 512])

    bcp = pool("bcast", 4)
    scanp = pool("scan", 2)

    # ---------------- dt_proj -> dt = -softplus(...) via ln(sigmoid(-x)) ----------------
    # (no Softplus table on gen3; sigmoid/ln batched over e-tile PAIRS to
    # halve the Act table reloads)
    dt = [big.tile([128, L], FP16, tag=f"dt{e}", name=f"dt{e}") for e in range(ET)]
    for ep in range(ET // 2):
        lntmps = {}
        for e in (2 * ep, 2 * ep + 1):
            lntmp = chunk.tile([128, L], F32, tag="lntmp", name=f"lntmp{e}")
            lntmps[e] = lntmp
            for t in range(TC):
                ps = psmall.tile([128, 512], F32, tag="mm", name="psmm")
                nc.tensor.matmul(
                    ps[:], w_dtT[:, e * 128:(e + 1) * 128],
                    xdblR[:, t * 512:(t + 1) * 512], start=True, stop=True,
                )
                nc.scalar.activation(lntmp[:, t * 512:(t + 1) * 512], ps[:], AF.Sigmoid,
                                     bias=dtb[e][:], scale=-1.0)
        for e in (2 * ep, 2 * ep + 1):
            for t in range(TC):
                nc.scalar.activation(dt[e][:, t * 512:(t + 1) * 512],
                                     lntmps[e][:, t * 512:(t + 1) * 512], AF.Ln)

    w = [big.tile([128, L], FP16, tag=f"w{e}", name=f"w{e}") for e in range(ET)]
    for e in range(ET):
        nc.vector.tensor_tensor(w[e][:], dt[e][:], u[e][:], MULT)

    # ---------------- selective scan ----------------
    # Concatenated scans: the tensor_tensor_scan instruction has a ~4us fixed
    # cost, so pack NCAT n-states into one long scan per (e, quad). State
    # resets at unit boundaries are free: h[0] = dA[0]*h[-1] + z[0] and the
    # boundary dA column is pinned to zero (Exp writes skip it; a per-set
    # memset clears it), so each unit starts from h=0 exactly.
    # n-outer over e-pairs: b/c broadcast once per (pass, n) instead of per
    # (e, n); PSUM holds two full-L f32 y-accumulators (8 banks).
    NCAT = 4
    LCAT = NCAT * L
    mmwp.release()
    psmall.release()
    pbig = pool("psum_y", 1, space="PSUM")       # 2x [128,2048] y accumulators
    yg = [None] * ET
    for pair in range(ET // 2):
        es = [2 * pair, 2 * pair + 1]
        py = {}
        for e in es:
            py[e] = pbig.tile([128, L], F32, tag=f"py{e - 2 * pair}", name=f"py{e}")
            for t in range(TC):
                nc.tensor.matmul(
                    py[e][:, t * 512:(t + 1) * 512], diag_dp[e][:],
                    u[e][:, t * 512:(t + 1) * 512],
                    start=True, stop=False,
                )
        for q in range(N // NCAT):
            ns = list(range(q * NCAT, (q + 1) * NCAT))
            bcs = []
            for n in ns:
                b_bc = bcp.tile([128, L], FP16, tag="bbc", name="bbc")
                c_bc = bcp.tile([128, L], FP16, tag="cbc", name="cbc")
                nc.sync.dma_start(b_bc[:], bc_dram[n, :].partition_broadcast(128))
                nc.sync.dma_start(c_bc[:], bc_dram[N + n, :].partition_broadcast(128))
                bcs.append((b_bc, c_bc))
            for e in es:
                dA = scanp.tile([128, LCAT], FP16, tag="dA", name="dA")
                z = scanp.tile([128, LCAT], FP16, tag="z", name="z")
                for k, n in enumerate(ns):
                    if k > 0:
                        nc.vector.memset(dA[:, k * L:k * L + 1], 0.0)
                    off = k * L + (1 if k > 0 else 0)
                    nc.scalar.activation(dA[:, off:(k + 1) * L],
                                         dt[e][:, off - k * L:L], AF.Exp,
                                         scale=aneg[e][:, n:n + 1])
                    # z slices all on DVE (Pool SBUF-port contention test)
                    nc.vector.tensor_tensor(z[:, k * L:(k + 1) * L], w[e][:],
                                            bcs[k][0][:], MULT)
                # in-place scan: h overwrites z (same-position write-behind)
                nc.vector.tensor_tensor_scan(z[:], dA[:], z[:], 0.0, MULT, ADD)
                # yp slices in-place over dA, then accumulate into py via PE
                for k, n in enumerate(ns):
                    nc.vector.tensor_tensor(dA[:, k * L:(k + 1) * L],
                                            z[:, k * L:(k + 1) * L],
                                            bcs[k][1][:], MULT)
                    for t in range(TC):
                        nc.tensor.matmul(
                            py[e][:, t * 512:(t + 1) * 512], identb[:],
                            dA[:, k * L + t * 512: k * L + (t + 1) * 512],
                            start=False, stop=(q == N // NCAT - 1 and k == NCAT - 1),
                        )
                # gating emitted per-e right after its accumulator closes, so
                # e0's gating overlaps e1's scan work in the final quad
                if q == N // NCAT - 1:
                    yge = big.tile([128, L], FP16, tag=f"u{e}", name=f"yg{e}")
                    yg[e] = yge
                    for t in range(TC):
                        zs_l = chunk.tile([128, 512], FP16, tag="zs_s", name="zs_l")
                        nc.sync.dma_start(
                            zs_l[:], zs_dram[e * 128:(e + 1) * 128, t * 512:(t + 1) * 512])
                        py_s = chunk.tile([128, 512], FP16, tag="py_s", name="py_s")
                        nc.scalar.copy(py_s[:], py[e][:, t * 512:(t + 1) * 512])
                        nc.vector.tensor_tensor(
                            yge[:, t * 512:(t + 1) * 512],
                            py_s[:],
                            zs_l[:], MULT,
                        )
    pbig.release()
    psout = pool("psum_out", 2, space="PSUM")

    # ---------------- out_proj + resid-add (via matmul) + mask + store ----------------
    for do in range(DT):
        pss = [psout.tile([128, 512], F32, tag=f"op{t}", bufs=1, name=f"op{do}_{t}")
               for t in range(TC)]
        for e in range(ET):
            for t in range(TC):
                nc.tensor.matmul(
                    pss[t][:], w_outT[e][:, do * 128:(do + 1) * 128],
                    yg[e][:, t * 512:(t + 1) * 512],
                    start=(e == 0), stop=False,
                )
        for t in range(TC):
            resid_l = chunk.tile([128, 512], FP16, tag="resid_s", name="resid_l")
            nc.sync.dma_start(
                resid_l[:], resid_dram[do * 128:(do + 1) * 128, t * 512:(t + 1) * 512])
            nc.tensor.matmul(
                pss[t][:], flagident[:],
                resid_l[:],
                start=False, stop=True,
            )
            ofin = chunk.tile([128, 512], F32, tag="ofin", name="ofin")
            nc.vector.tensor_tensor(ofin[:], pss[t][:], maskb[:, t * 512:(t + 1) * 512], MULT)
            nc.sync.dma_start(out_d[do * 128:(do + 1) * 128, t * 512:(t + 1) * 512], ofin[:])

    for p in reversed(_pools):
        if not p._released:
            p.release()


_NC_CACHE = {}


def _get_nc():
    if "nc" not in _NC_CACHE:
        _NC_CACHE["nc"] = build_nc()
    return _NC_CACHE["nc"]


def _core_inputs(blk, b, inputs):
    pfx = "s_" if blk == 0 else "c_"
    xin = inputs["x"][b] if blk == 0 else inputs["encoder_states"][b]
    f = 1.0 if blk == 0 else 0.0
    g = lambda k: np.asarray(inputs[pfx + k])
    aneg = np.exp(g("A_log"))  # = -A; dt tile holds -softplus so dA = exp(aneg*dt)
    return {
        "xin": np.ascontiguousarray(xin, np.float32),
        "maskb": np.ascontiguousarray(
            np.broadcast_to(inputs["padding_mask"][b][None, :], (128, L))).astype(np.float16),
        "flagv": np.full((128, 1), f, np.float32),
        "flag2": np.full((128, 1), 1.0 - f, np.float32),
        "ffw": np.ascontiguousarray(np.asarray(inputs["ff_w"]).transpose(2, 1, 0)).astype(np.float16),
        "ffb": np.asarray(inputs["ff_b"]).reshape(D, 1).astype(np.float32),
        "w_inT": np.ascontiguousarray(g("in_proj_w").T).astype(np.float16),
        "cw": np.ascontiguousarray(g("conv_w").reshape(E, K), np.float32),
        "cb": g("conv_b").reshape(E, 1).astype(np.float32),
        "w_xT": np.ascontiguousarray(
            g("x_proj_w").T[:, list(range(R, R + 2 * N)) + list(range(R))]
        ).astype(np.float16),
        "w_dtT": np.ascontiguousarray(g("dt_proj_w").T).astype(np.float16),
        "dtb": (-g("dt_proj_b")).reshape(E, 1).astype(np.float32),
        "aneg": np.ascontiguousarray(aneg, np.float32),
        "dp": (-g("D")).reshape(E, 1).astype(np.float32),
        "w_outT": np.ascontiguousarray(-g("out_proj_w").T).astype(np.float16),
    }


def kernel(**inputs):
    nc = _get_nc()
    in_maps = []
    for b in range(B):
        in_maps.append(_core_inputs(0, b, inputs))  # core 2b: self block
        in_maps.append(_core_inputs(1, b, inputs))  # core 2b+1: cross block
    res = run_bass_kernel_spmd(nc, in_maps, core_ids=list(range(8)))
    out = np.empty((B, D, L), np.float32)
    for b in range(B):
        out[b] = res.results[2 * b]["out"] + res.results[2 * b + 1]["out"]
    return out



# revision 5
# speedup vs baseline: 1.0166x; 1.0053x over previous
"""Trainium2 Bass kernel for nn_AttModule_mamba_cross (B=4,D=256,L=2048,E=512,N=16,K=7,R=16).

Sharding: 8 cores = 2 mamba blocks x 4 batches, one (block, batch) unit per core.
All cores run one SPMD program; a per-core flag selects whether the conv_ff(x)
path is blended in (self-block cores) or the raw input is used (cross-block
cores). Host sums each core pair's partial outputs.

DVE-centric design (the scan engine is the bottleneck; measured costs drove
every choice):
- The selective scan runs as 16 concatenated tensor_tensor_scan instructions
  ([128, 4*2048]): 4 n-states share one scan, with the boundary dA column
  pinned to 0 so the recurrence state resets exactly between units.
- All elementwise muls (z = w*b, yp = h*c) stay on the DVE: GpSimd shares
  SBUF ports with the DVE and concurrent Pool work slows DVE tensor ops
  more than it helps.
- n-outer over e-tile pairs: B/C rows are broadcast once per (pass, n) via
  DMA instead of once per (e, n); PSUM holds two full-L f32 y-accumulators.
- resid/zs spill to DRAM to make room for the wide scan tiles; gating
  evacuates PSUM through the Act engine so the DVE mul runs at the 2x fp16
  rate; sigmoid/ln run batched per e-pair to halve Act table reloads.
padding_mask is all-ones per the problem spec (fill: ones); it is applied
only at the final output store.
"""
import numpy as np

import concourse.bass as bass
import concourse.bacc as bacc
import concourse.mybir as mybir
import concourse.tile as tile
from concourse import masks
from concourse.bass_utils import run_bass_kernel_spmd

B, D, L = 4, 256, 2048
E, N, K = 512, 16, 7
R = 16
EPS = 1e-5

F32 = mybir.dt.float32
FP16 = mybir.dt.float16
MULT = mybir.AluOpType.mult
ADD = mybir.AluOpType.add
SUB = mybir.AluOpType.subtract
AF = mybir.ActivationFunctionType

DT = D // 128   # 2 d-tiles
ET = E // 128   # 4 e-tiles
TC = L // 512   # 4 t-chunks of 512


def build_nc():
    nc = bacc.Bacc("TRN2", target_bir_lowering=False, debug=False, num_devices=8)

    # ---- DRAM I/O ----
    xin_d = nc.dram_tensor("xin", [D, L], F32, kind="ExternalInput")
    maskb_d = nc.dram_tensor("maskb", [128, L], FP16, kind="ExternalInput")
    flagv_d = nc.dram_tensor("flagv", [128, 1], F32, kind="ExternalInput")
    flag2_d = nc.dram_tensor("flag2", [128, 1], F32, kind="ExternalInput")
    ffw_d = nc.dram_tensor("ffw", [K, D, D], FP16, kind="ExternalInput")
    ffb_d = nc.dram_tensor("ffb", [D, 1], F32, kind="ExternalInput")
    w_inT_d = nc.dram_tensor("w_inT", [D, 2 * E], FP16, kind="ExternalInput")
    cw_d = nc.dram_tensor("cw", [E, K], F32, kind="ExternalInput")
    cb_d = nc.dram_tensor("cb", [E, 1], F32, kind="ExternalInput")
    w_xT_d = nc.dram_tensor("w_xT", [E, R + 2 * N], FP16, kind="ExternalInput")
    w_dtT_d = nc.dram_tensor("w_dtT", [R, E], FP16, kind="ExternalInput")
    dtb_d = nc.dram_tensor("dtb", [E, 1], F32, kind="ExternalInput")
    aneg_d = nc.dram_tensor("aneg", [E, N], F32, kind="ExternalInput")
    dp_d = nc.dram_tensor("dp", [E, 1], F32, kind="ExternalInput")
    w_outT_d = nc.dram_tensor("w_outT", [E, D], FP16, kind="ExternalInput")
    out_d = nc.dram_tensor("out", [D, L], F32, kind="ExternalOutput")

    with tile.TileContext(nc) as tc:
        _emit(nc, tc, locals())
    nc.compile()
    return nc


def _emit(nc, tc, d):
    xin_d, maskb_d, flagv_d, flag2_d = d["xin_d"], d["maskb_d"], d["flagv_d"], d["flag2_d"]
    ffw_d, ffb_d, w_inT_d = d["ffw_d"], d["ffb_d"], d["w_inT_d"]
    cw_d, cb_d, w_xT_d, w_dtT_d = d["cw_d"], d["cb_d"], d["w_xT_d"], d["w_dtT_d"]
    dtb_d, aneg_d, dp_d, w_outT_d, out_d = d["dtb_d"], d["aneg_d"], d["dp_d"], d["w_outT_d"], d["out_d"]

    _pools = []

    def pool(name, bufs, space="SBUF"):
        p = tc.alloc_tile_pool(name=name, bufs=bufs, space=space)
        _pools.append(p)
        return p

    wpool = pool("weights", 1)          # persistent small weights/constants
    big = pool("big", 1)                # persistent big activations
    chunk = pool("chunk512", 2)         # transient [128,512] tiles
    psmall = pool("psum_mm", 2, space="PSUM")    # [128,512] matmul tiles
    mmwp = pool("psum_mmw", 2, space="PSUM")     # [128,512] prelude matmul tiles
    dramp = pool("dram", 1, space="DRAM")
    # stage pools, created in reverse order of release (stack allocator is LIFO)
    stage2 = pool("stage2", 1)          # xp halo + diag_cw (released after dwconv)
    stage3 = pool("stage3", 1)          # mamba_in (released after gate)
    wff = pool("wff", 1)                # conv_ff weights + xin16 + convout (released after norm)
    stage1 = pool("stage1", 1)          # xin f32 (released after cast)

    # ---------------- stage1: input load + fp16 cast (issued first so the
    # conv_ff critical path starts as early as possible) ----------------
    xin = [stage1.tile([128, L], F32, tag=f"xinf{i}", name=f"xinf{i}") for i in range(DT)]
    for i in range(DT):
        nc.sync.dma_start(xin[i][:], xin_d[i * 128:(i + 1) * 128, :])
    ffw = [wff.tile([128, K * D], FP16, tag=f"ffw{i}", name=f"ffw{i}") for i in range(DT)]
    for i in range(DT):
        nc.sync.dma_start(
            ffw[i][:].rearrange("p (k d) -> p k d", k=K),
            ffw_d[:, i * 128:(i + 1) * 128, :].rearrange("k p d -> p k d"),
        )
    # padded fp16 input (3 zeros each side for the K=7 same-padding conv)
    xin16 = [wff.tile([128, L + 6], FP16, tag=f"xin16_{i}", name=f"xin16_{i}") for i in range(DT)]
    for i in range(DT):
        nc.gpsimd.memset(xin16[i][:, 0:3], 0.0)
        nc.gpsimd.memset(xin16[i][:, L + 3:], 0.0)
        nc.vector.tensor_copy(xin16[i][:, 3:L + 3], xin[i][:])
    stage1.release()

    # ---------------- persistent weights ----------------
    maskb = wpool.tile([128, L], FP16, tag="maskb", name="maskb")
    nc.sync.dma_start(maskb[:], maskb_d[:])
    flagv = wpool.tile([128, 1], F32, tag="flagv", name="flagv")
    flag2 = wpool.tile([128, 1], F32, tag="flag2", name="flag2")
    nc.sync.dma_start(flagv[:], flagv_d[:])
    nc.sync.dma_start(flag2[:], flag2_d[:])
    ffb = [wpool.tile([128, 1], F32, tag=f"ffb{i}", name=f"ffb{i}") for i in range(DT)]
    for i in range(DT):
        nc.sync.dma_start(ffb[i][:], ffb_d[i * 128:(i + 1) * 128, :])
    w_inT = [wpool.tile([128, 2 * E], FP16, tag=f"w_inT{i}", name=f"w_inT{i}") for i in range(DT)]
    for i in range(DT):
        nc.sync.dma_start(w_inT[i][:], w_inT_d[i * 128:(i + 1) * 128, :])
    cw = [wpool.tile([128, K], F32, tag=f"cw{i}", name=f"cw{i}") for i in range(ET)]
    cb = [wpool.tile([128, 1], F32, tag=f"cb{i}", name=f"cb{i}") for i in range(ET)]
    dtb = [wpool.tile([128, 1], F32, tag=f"dtb{i}", name=f"dtb{i}") for i in range(ET)]
    dp = [wpool.tile([128, 1], F32, tag=f"dp{i}", name=f"dp{i}") for i in range(ET)]
    aneg = [wpool.tile([128, N], F32, tag=f"aneg{i}", name=f"aneg{i}") for i in range(ET)]
    w_xT = [wpool.tile([128, R + 2 * N], FP16, tag=f"w_xT{i}", name=f"w_xT{i}") for i in range(ET)]
    w_outT = [wpool.tile([128, D], FP16, tag=f"w_outT{i}", name=f"w_outT{i}") for i in range(ET)]
    for i in range(ET):
        sl = slice(i * 128, (i + 1) * 128)
        nc.sync.dma_start(cw[i][:], cw_d[sl, :])
        nc.sync.dma_start(cb[i][:], cb_d[sl, :])
        nc.sync.dma_start(dtb[i][:], dtb_d[sl, :])
        nc.sync.dma_start(dp[i][:], dp_d[sl, :])
        nc.sync.dma_start(aneg[i][:], aneg_d[sl, :])
        nc.sync.dma_start(w_xT[i][:], w_xT_d[sl, :])
        nc.sync.dma_start(w_outT[i][:], w_outT_d[sl, :])
    w_dtT = wpool.tile([R, E], FP16, tag="w_dtT", name="w_dtT")
    nc.sync.dma_start(w_dtT[:], w_dtT_d[:])

    ident = wpool.tile([128, 128], F32, tag="ident", name="ident")
    masks.make_identity(nc, ident[:])
    identb = wpool.tile([128, 128], FP16, tag="identb", name="identb")
    nc.vector.tensor_copy(identb[:], ident[:])
    # flag-scaled identity: adds resid into the out_proj psum on self cores only
    flagident = wpool.tile([128, 128], FP16, tag="flagident", name="flagident")
    nc.vector.tensor_scalar_mul(flagident[:], ident[:], flagv[:])
    diag_dp = [wpool.tile([128, 128], FP16, tag=f"ddp{e}", name=f"ddp{e}") for e in range(ET)]
    for e in range(ET):
        nc.vector.tensor_scalar_mul(diag_dp[e][:], ident[:], dp[e][:])
    diag_cw = [[stage2.tile([128, 128], FP16, tag=f"dcw{e}_{k}", name=f"dcw{e}_{k}") for k in range(K)]
               for e in range(ET)]
    for e in range(ET):
        for k in range(K):
            nc.vector.tensor_scalar_mul(diag_cw[e][k][:], ident[:], cw[e][:, k:k + 1])

    # ---------------- conv_ff -> convout (fp16) + resid (fp16) ----------------
    convout = [wff.tile([128, L], FP16, tag=f"convout{i}", name=f"convout{i}") for i in range(DT)]
    resid_dram = dramp.tile([D, L], FP16, tag="residd", name="residd")
    for do in range(DT):
        for t in range(TC):
            ps = mmwp.tile([128, 512], F32, tag="mmw", name="psmmw")
            nmm = K * DT
            i = 0
            for k in range(K):
                for di in range(DT):
                    nc.tensor.matmul(
                        ps[:],
                        ffw[di][:, k * D + do * 128: k * D + (do + 1) * 128],
                        xin16[di][:, t * 512 + k: t * 512 + k + 512],
                        start=(i == 0), stop=(i == nmm - 1),
                    )
                    i += 1
            nc.scalar.activation(convout[do][:, t * 512:(t + 1) * 512], ps[:], AF.Relu, bias=ffb[do][:])
            resid_s = chunk.tile([128, 512], FP16, tag="resid_s", name="resid_s")
            nc.vector.tensor_tensor(
                resid_s[:],
                convout[do][:, t * 512:(t + 1) * 512],
                xin16[do][:, 3 + t * 512: 3 + (t + 1) * 512], ADD,
            )
            nc.sync.dma_start(
                resid_dram[do * 128:(do + 1) * 128, t * 512:(t + 1) * 512], resid_s[:])

    # ---------------- instance norm -> mamba_in (fp16) ----------------
    # norm input: self cores = convout, cross cores = xin  (flagv selects)
    mamba_in = [stage3.tile([128, L], FP16, tag=f"mambain{i}", name=f"mambain{i}") for i in range(DT)]
    for i in range(DT):
        nin = chunk.tile([128, L], FP16, tag="nin2048", name="nin")
        ssum = wpool.tile([128, 1], F32, tag=f"ssum{i}", name=f"ssum{i}")
        ssq = wpool.tile([128, 1], F32, tag=f"ssq{i}", name=f"ssq{i}")
        nc.vector.tensor_scalar_mul(nin[:], xin16[i][:, 3:L + 3], flag2[:])
        nc.vector.scalar_tensor_tensor(nin[:], convout[i][:], flagv[:], nin[:], MULT, ADD,
                                       accum_out=ssum[:])
        trash = chunk.tile([128, L], FP16, tag="nin2048", name="trash")
        nc.scalar.activation(trash[:], nin[:], AF.Square, accum_out=ssq[:])
        mean = wpool.tile([128, 1], F32, tag=f"mean{i}", name=f"mean{i}")
        var = wpool.tile([128, 1], F32, tag=f"var{i}", name=f"var{i}")
        nc.vector.tensor_scalar_mul(mean[:], ssum[:], 1.0 / L)
        nc.vector.tensor_scalar_mul(var[:], ssq[:], 1.0 / L)
        msq = wpool.tile([128, 1], F32, tag=f"msq{i}", name=f"msq{i}")
        nc.vector.tensor_tensor(msq[:], mean[:], mean[:], MULT)
        nc.vector.scalar_tensor_tensor(var[:], msq[:], -1.0, var[:], MULT, ADD)
        nc.vector.tensor_scalar_add(var[:], var[:], EPS)
        inv = wpool.tile([128, 1], F32, tag=f"inv{i}", name=f"inv{i}")
        nc.vector.reciprocal(inv[:], var[:])
        nc.scalar.sqrt(inv[:], inv[:])
        muinv = wpool.tile([128, 1], F32, tag=f"muinv{i}", name=f"muinv{i}")
        nc.vector.tensor_tensor(muinv[:], mean[:], inv[:], MULT)
        nc.vector.tensor_scalar(mamba_in[i][:], nin[:], inv[:], muinv[:], MULT, SUB)
    wff.release()

    # ---------------- in_proj (xp half) ----------------
    xp = [stage2.tile([128, L + 6], FP16, tag=f"xp{e}", name=f"xp{e}") for e in range(ET)]
    for e in range(ET):
        nc.gpsimd.memset(xp[e][:, 0:6], 0.0)
        for t in range(TC):
            ps = mmwp.tile([128, 512], F32, tag="mmw", name="psmmw")
            for di in range(DT):
                nc.tensor.matmul(
                    ps[:], w_inT[di][:, e * 128:(e + 1) * 128],
                    mamba_in[di][:, t * 512:(t + 1) * 512],
                    start=(di == 0), stop=(di == DT - 1),
                )
            nc.vector.tensor_copy(xp[e][:, 6 + t * 512: 6 + (t + 1) * 512], ps[:])

    # ---------------- gate half: zs = silu(in_proj_z), spilled to DRAM ----------------
    zs_dram = dramp.tile([E, L], FP16, tag="zsd", name="zsd")
    for e in range(ET):
        for t in range(TC):
            ps = psmall.tile([128, 512], F32, tag="mm", name="psmm")
            for di in range(DT):
                nc.tensor.matmul(
                    ps[:], w_inT[di][:, E + e * 128: E + (e + 1) * 128],
                    mamba_in[di][:, t * 512:(t + 1) * 512],
                    start=(di == 0), stop=(di == DT - 1),
                )
            zs_s = chunk.tile([128, 512], FP16, tag="zs_s", name="zs_s")
            nc.scalar.activation(zs_s[:], ps[:], AF.Silu)
            nc.sync.dma_start(
                zs_dram[e * 128:(e + 1) * 128, t * 512:(t + 1) * 512], zs_s[:])
    stage3.release()

    # ---------------- depthwise causal conv + silu -> u ----------------
    u = [big.tile([128, L], FP16, tag=f"u{e}", name=f"u{e}") for e in range(ET)]
    for e in range(ET):
        for t in range(TC):
            ps = mmwp.tile([128, 512], F32, tag="mmw", name="psmmw")
            for k in range(K):
                nc.tensor.matmul(
                    ps[:], diag_cw[e][k][:],
                    xp[e][:, t * 512 + k: t * 512 + k + 512],
                    start=(k == 0), stop=(k == K - 1),
                )
            nc.scalar.activation(u[e][:, t * 512:(t + 1) * 512], ps[:], AF.Silu, bias=cb[e][:])
    stage2.release()

    # ---------------- x_proj -> xdblR (fp16) + B/C rows to DRAM ----------------
    xdblR = big.tile([R, L], FP16, tag="xdblR", name="xdblR")
    bcsrc = big.tile([2 * N, L], FP16, tag="bcsrc", name="bcsrc")
    for t in range(TC):
        ps = mmwp.tile([R + 2 * N, 512], F32, tag="mmw", name="psmmx")
        for e in range(ET):
            nc.tensor.matmul(
                ps[:], w_xT[e][:], u[e][:, t * 512:(t + 1) * 512],
                start=(e == 0), stop=(e == ET - 1),
            )
        nc.scalar.copy(bcsrc[:, t * 512:(t + 1) * 512], ps[0:2 * N, :])
        nc.scalar.copy(xdblR[:, t * 512:(t + 1) * 512], ps[2 * N:2 * N + R, :])
    bc_dram = dramp.tile([2 * N, L], FP16, tag="bcdram", name="bcdram")
    for t in range(TC):
        nc.sync.dma_start(bc_dram[:, t * 512:(t + 1) * 512],
                          bcsrc[:, t * 512:(t + 1) * 512])

    bcp = pool("bcast", 4)
    scanp = pool("scan", 2)

    # ---------------- dt_proj -> dt = -softplus(...) via ln(sigmoid(-x)) ----------------
    # (no Softplus table on gen3; sigmoid/ln batched over e-tile PAIRS to
    # halve the Act table reloads)
    dt = [big.tile([128, L], FP16, tag=f"dt{e}", name=f"dt{e}") for e in range(ET)]
    for ep in range(ET // 2):
        lntmps = {}
        for e in (2 * ep, 2 * ep + 1):
            lntmp = chunk.tile([128, L], F32, tag="lntmp", name=f"lntmp{e}")
            lntmps[e] = lntmp
            for t in range(TC):
                ps = psmall.tile([128, 512], F32, tag="mm", name="psmm")
                nc.tensor.matmul(
                    ps[:], w_dtT[:, e * 128:(e + 1) * 128],
                    xdblR[:, t * 512:(t + 1) * 512], start=True, stop=True,
                )
                nc.scalar.activation(lntmp[:, t * 512:(t + 1) * 512], ps[:], AF.Sigmoid,
                                     bias=dtb[e][:], scale=-1.0)
        for e in (2 * ep, 2 * ep + 1):
            for t in range(TC):
                nc.scalar.activation(dt[e][:, t * 512:(t + 1) * 512],
                                     lntmps[e][:, t * 512:(t + 1) * 512], AF.Ln)

    w = [big.tile([128, L], FP16, tag=f"w{e}", name=f"w{e}") for e in range(ET)]
    for e in range(ET):
        nc.vector.tensor_tensor(w[e][:], dt[e][:], u[e][:], MULT)

    # ---------------- selective scan ----------------
    # Concatenated scans: the tensor_tensor_scan instruction has a ~4us fixed
    # cost, so pack NCAT n-states into one long scan per (e, quad). State
    # resets at unit boundaries are free: h[0] = dA[0]*h[-1] + z[0] and the
    # boundary dA column is pinned to zero (Exp writes skip it; a per-set
    # memset clears it), so each unit starts from h=0 exactly.
    # n-outer over e-pairs: b/c broadcast once per (pass, n) instead of per
    # (e, n); PSUM holds two full-L f32 y-accumulators (8 banks).
    NCAT = 4
    LCAT = NCAT * L
    mmwp.release()
    psmall.release()
    pbig = pool("psum_y", 1, space="PSUM")       # 2x [128,2048] y accumulators
    yg = [None] * ET
    for pair in range(ET // 2):
        es = [2 * pair, 2 * pair + 1]
        py = {}
        for e in es:
            py[e] = pbig.tile([128, L], F32, tag=f"py{e - 2 * pair}", name=f"py{e}")
            for t in range(TC):
                nc.tensor.matmul(
                    py[e][:, t * 512:(t + 1) * 512], diag_dp[e][:],
                    u[e][:, t * 512:(t + 1) * 512],
                    start=True, stop=False,
                )
        for q in range(N // NCAT):
            ns = list(range(q * NCAT, (q + 1) * NCAT))
            bcs = []
            for n in ns:
                b_bc = bcp.tile([128, L], FP16, tag="bbc", name="bbc")
                c_bc = bcp.tile([128, L], FP16, tag="cbc", name="cbc")
                nc.sync.dma_start(b_bc[:], bc_dram[n, :].partition_broadcast(128))
                nc.sync.dma_start(c_bc[:], bc_dram[N + n, :].partition_broadcast(128))
                bcs.append((b_bc, c_bc))
            for e in es:
                dA = scanp.tile([128, LCAT], FP16, tag="dA", name="dA")
                z = scanp.tile([128, LCAT], FP16, tag="z", name="z")
                for k, n in enumerate(ns):
                    if k > 0:
                        nc.vector.memset(dA[:, k * L:k * L + 1], 0.0)
                    off = k * L + (1 if k > 0 else 0)
                    nc.scalar.activation(dA[:, off:(k + 1) * L],
                                         dt[e][:, off - k * L:L], AF.Exp,
                                         scale=aneg[e][:, n:n + 1])
                    # z slices all on DVE (Pool SBUF-port contention test)
                    nc.vector.tensor_tensor(z[:, k * L:(k + 1) * L], w[e][:],
                                            bcs[k][0][:], MULT)
                # in-place scan: h overwrites z (same-position write-behind)
                nc.vector.tensor_tensor_scan(z[:], dA[:], z[:], 0.0, MULT, ADD)
                # yp slices in-place over dA, then accumulate into py via PE
                for k, n in enumerate(ns):
                    nc.vector.tensor_tensor(dA[:, k * L:(k + 1) * L],
                                            z[:, k * L:(k + 1) * L],
                                            bcs[k][1][:], MULT)
                    for t in range(TC):
                        nc.tensor.matmul(
                            py[e][:, t * 512:(t + 1) * 512], identb[:],
                            dA[:, k * L + t * 512: k * L + (t + 1) * 512],
                            start=False, stop=(q == N // NCAT - 1 and k == NCAT - 1),
                        )
                # gating emitted per-e right after its accumulator closes, so
                # e0's gating overlaps e1's scan work in the final quad
                if q == N // NCAT - 1:
                    yge = big.tile([128, L], FP16, tag=f"u{e}", name=f"yg{e}")
                    yg[e] = yge
                    for t in range(TC):
                        zs_l = chunk.tile([128, 512], FP16, tag="zs_s", name="zs_l")
                        nc.sync.dma_start(
                            zs_l[:], zs_dram[e * 128:(e + 1) * 128, t * 512:(t + 1) * 512])
                        py_s = chunk.tile([128, 512], FP16, tag="py_s", name="py_s")
                        nc.scalar.copy(py_s[:], py[e][:, t * 512:(t + 1) * 512])
                        nc.vector.tensor_tensor(
                            yge[:, t * 512:(t + 1) * 512],
                            py_s[:],
                            zs_l[:], MULT,
                        )
    pbig.release()
    psout = pool("psum_out", 2, space="PSUM")

    # ---------------- out_proj + resid-add (via matmul) + mask + store ----------------
    for do in range(DT):
        pss = [psout.tile([128, 512], F32, tag=f"op{t}", bufs=1, name=f"op{do}_{t}")
               for t in range(TC)]
        for e in range(ET):
            for t in range(TC):
                nc.tensor.matmul(
                    pss[t][:], w_outT[e][:, do * 128:(do + 1) * 128],
                    yg[e][:, t * 512:(t + 1) * 512],
                    start=(e == 0), stop=False,
                )
        for t in range(TC):
            resid_l = chunk.tile([128, 512], FP16, tag="resid_s", name="resid_l")
            nc.sync.dma_start(
                resid_l[:], resid_dram[do * 128:(do + 1) * 128, t * 512:(t + 1) * 512])
            nc.tensor.matmul(
                pss[t][:], flagident[:],
                resid_l[:],
                start=False, stop=True,
            )
            ofin = chunk.tile([128, 512], F32, tag="ofin", name="ofin")
            nc.vector.tensor_tensor(ofin[:], pss[t][:], maskb[:, t * 512:(t + 1) * 512], MULT)
            nc.sync.dma_start(out_d[do * 128:(do + 1) * 128, t * 512:(t + 1) * 512], ofin[:])

    for p in reversed(_pools):
        if not p._released:
            p.release()


_NC_CACHE = {}


def _get_nc():
    if "nc" not in _NC_CACHE:
        _NC_CACHE["nc"] = build_nc()
    return _NC_CACHE["nc"]


def _core_inputs(blk, b, inputs):
    pfx = "s_" if blk == 0 else "c_"
    xin = inputs["x"][b] if blk == 0 else inputs["encoder_states"][b]
    f = 1.0 if blk == 0 else 0.0
    g = lambda k: np.asarray(inputs[pfx + k])
    aneg = np.exp(g("A_log"))  # = -A; dt tile holds -softplus so dA = exp(aneg*dt)
    return {
        "xin": np.ascontiguousarray(xin, np.float32),
        "maskb": np.ascontiguousarray(
            np.broadcast_to(inputs["padding_mask"][b][None, :], (128, L))).astype(np.float16),
        "flagv": np.full((128, 1), f, np.float32),
        "flag2": np.full((128, 1), 1.0 - f, np.float32),
        "ffw": np.ascontiguousarray(np.asarray(inputs["ff_w"]).transpose(2, 1, 0)).astype(np.float16),
        "ffb": np.asarray(inputs["ff_b"]).reshape(D, 1).astype(np.float32),
        "w_inT": np.ascontiguousarray(g("in_proj_w").T).astype(np.float16),
        "cw": np.ascontiguousarray(g("conv_w").reshape(E, K), np.float32),
        "cb": g("conv_b").reshape(E, 1).astype(np.float32),
        "w_xT": np.ascontiguousarray(
            g("x_proj_w").T[:, list(range(R, R + 2 * N)) + list(range(R))]
        ).astype(np.float16),
        "w_dtT": np.ascontiguousarray(g("dt_proj_w").T).astype(np.float16),
        "dtb": (-g("dt_proj_b")).reshape(E, 1).astype(np.float32),
        "aneg": np.ascontiguousarray(aneg, np.float32),
        "dp": (-g("D")).reshape(E, 1).astype(np.float32),
        "w_outT": np.ascontiguousarray(-g("out_proj_w").T).astype(np.float16),
    }


def kernel(**inputs):
    nc = _get_nc()
    in_maps = []
    for b in range(B):
        in_maps.append(_core_inputs(0, b, inputs))  # core 2b: self block
        in_maps.append(_core_inputs(1, b, inputs))  # core 2b+1: cross block
    res = run_bass_kernel_spmd(nc, in_maps, core_ids=list(range(8)))
    out = np.empty((B, D, L), np.float32)
    for b in range(B):
        out[b] = res.results[2 * b]["out"] + res.results[2 * b + 1]["out"]
    return out



# revision 7
# speedup vs baseline: 1.0278x; 1.0111x over previous
"""Trainium2 Bass kernel for nn_AttModule_mamba_cross (B=4,D=256,L=2048,E=512,N=16,K=7,R=16).

Sharding: 8 cores = 2 mamba blocks x 4 batches, one (block, batch) unit per core.
All cores run one SPMD program; a per-core flag selects whether the conv_ff(x)
path is blended in (self-block cores) or the raw input is used (cross-block
cores). Host sums each core pair's partial outputs.

DVE-centric design (the scan engine is the bottleneck; every choice below is
backed by hardware measurement):
- The selective scan runs as 16 concatenated tensor_tensor_scan instructions
  ([128, 4*2048]): 4 n-states share one scan, with each unit's boundary dA
  column pinned to 0 so the recurrence state resets exactly between units.
- All elementwise muls (z = w*b, yp = h*c) stay on the DVE: GpSimd shares
  SBUF ports with the DVE, so any concurrent Pool work slows DVE tensor ops
  more than the offload helps.
- n-outer over e-tile pairs: B/C rows are broadcast once per (pass, n) via
  DMA instead of once per (e, n); PSUM holds two full-L f32 y-accumulators.
- resid/zs spill to DRAM to make room for the wide scan tiles; gating is
  inlined per-e in the final quad with the PSUM evacuated through Act so
  the DVE mul runs at the 2x fp16 rate; sigmoid/ln run batched per e-pair
  to halve Act table reloads; the norm row-sum is fused into the flag-blend
  STT's accumulator.
- Pair-1's dt_proj raw output is DVE-staged into the (then unused) w[2]/w[3]
  tiles in the front; its sigmoid/ln and w-muls are deferred until after
  pass-0's first quad, so the first scan exponentials do not wait behind
  pair-1's activation chain.
padding_mask is all-ones per the problem spec (fill: ones); it is applied
only at the final output store.
"""
import numpy as np

import concourse.bass as bass
import concourse.bacc as bacc
import concourse.mybir as mybir
import concourse.tile as tile
from concourse import masks
from concourse.bass_utils import run_bass_kernel_spmd

B, D, L = 4, 256, 2048
E, N, K = 512, 16, 7
R = 16
EPS = 1e-5

F32 = mybir.dt.float32
FP16 = mybir.dt.float16
MULT = mybir.AluOpType.mult
ADD = mybir.AluOpType.add
SUB = mybir.AluOpType.subtract
AF = mybir.ActivationFunctionType

DT = D // 128   # 2 d-tiles
ET = E // 128   # 4 e-tiles
TC = L // 512   # 4 t-chunks of 512


def build_nc():
    nc = bacc.Bacc("TRN2", target_bir_lowering=False, debug=False, num_devices=8)

    # ---- DRAM I/O ----
    xin_d = nc.dram_tensor("xin", [D, L], F32, kind="ExternalInput")
    maskb_d = nc.dram_tensor("maskb", [128, L], FP16, kind="ExternalInput")
    flagv_d = nc.dram_tensor("flagv", [128, 1], F32, kind="ExternalInput")
    flag2_d = nc.dram_tensor("flag2", [128, 1], F32, kind="ExternalInput")
    ffw_d = nc.dram_tensor("ffw", [K, D, D], FP16, kind="ExternalInput")
    ffb_d = nc.dram_tensor("ffb", [D, 1], F32, kind="ExternalInput")
    w_inT_d = nc.dram_tensor("w_inT", [D, 2 * E], FP16, kind="ExternalInput")
    cw_d = nc.dram_tensor("cw", [E, K], F32, kind="ExternalInput")
    cb_d = nc.dram_tensor("cb", [E, 1], F32, kind="ExternalInput")
    w_xT_d = nc.dram_tensor("w_xT", [E, R + 2 * N], FP16, kind="ExternalInput")
    w_dtT_d = nc.dram_tensor("w_dtT", [R, E], FP16, kind="ExternalInput")
    dtb_d = nc.dram_tensor("dtb", [E, 1], F32, kind="ExternalInput")
    aneg_d = nc.dram_tensor("aneg", [E, N], F32, kind="ExternalInput")
    dp_d = nc.dram_tensor("dp", [E, 1], F32, kind="ExternalInput")
    w_outT_d = nc.dram_tensor("w_outT", [E, D], FP16, kind="ExternalInput")
    out_d = nc.dram_tensor("out", [D, L], F32, kind="ExternalOutput")

    with tile.TileContext(nc) as tc:
        _emit(nc, tc, locals())
    nc.compile()
    return nc


def _emit(nc, tc, d):
    xin_d, maskb_d, flagv_d, flag2_d = d["xin_d"], d["maskb_d"], d["flagv_d"], d["flag2_d"]
    ffw_d, ffb_d, w_inT_d = d["ffw_d"], d["ffb_d"], d["w_inT_d"]
    cw_d, cb_d, w_xT_d, w_dtT_d = d["cw_d"], d["cb_d"], d["w_xT_d"], d["w_dtT_d"]
    dtb_d, aneg_d, dp_d, w_outT_d, out_d = d["dtb_d"], d["aneg_d"], d["dp_d"], d["w_outT_d"], d["out_d"]

    _pools = []

    def pool(name, bufs, space="SBUF"):
        p = tc.alloc_tile_pool(name=name, bufs=bufs, space=space)
        _pools.append(p)
        return p

    wpool = pool("weights", 1)          # persistent small weights/constants
    big = pool("big", 1)                # persistent big activations
    chunk = pool("chunk512", 2)         # transient [128,512] tiles
    psmall = pool("psum_mm", 2, space="PSUM")    # [128,512] matmul tiles
    mmwp = pool("psum_mmw", 2, space="PSUM")     # [128,512] prelude matmul tiles
    dramp = pool("dram", 1, space="DRAM")
    # stage pools, created in reverse order of release (stack allocator is LIFO)
    stage2 = pool("stage2", 1)          # xp halo + diag_cw (released after dwconv)
    stage3 = pool("stage3", 1)          # mamba_in (released after gate)
    wff = pool("wff", 1)                # conv_ff weights + xin16 + convout (released after norm)
    stage1 = pool("stage1", 1)          # xin f32 (released after cast)

    # ---------------- stage1: input load + fp16 cast (issued first so the
    # conv_ff critical path starts as early as possible) ----------------
    xin = [stage1.tile([128, L], F32, tag=f"xinf{i}", name=f"xinf{i}") for i in range(DT)]
    for i in range(DT):
        nc.sync.dma_start(xin[i][:], xin_d[i * 128:(i + 1) * 128, :])
    ffw = [wff.tile([128, K * D], FP16, tag=f"ffw{i}", name=f"ffw{i}") for i in range(DT)]
    for i in range(DT):
        nc.sync.dma_start(
            ffw[i][:].rearrange("p (k d) -> p k d", k=K),
            ffw_d[:, i * 128:(i + 1) * 128, :].rearrange("k p d -> p k d"),
        )
    # padded fp16 input (3 zeros each side for the K=7 same-padding conv)
    xin16 = [wff.tile([128, L + 6], FP16, tag=f"xin16_{i}", name=f"xin16_{i}") for i in range(DT)]
    for i in range(DT):
        nc.gpsimd.memset(xin16[i][:, 0:3], 0.0)
        nc.gpsimd.memset(xin16[i][:, L + 3:], 0.0)
        nc.vector.tensor_copy(xin16[i][:, 3:L + 3], xin[i][:])
    stage1.release()

    # ---------------- persistent weights ----------------
    maskb = wpool.tile([128, L], FP16, tag="maskb", name="maskb")
    nc.sync.dma_start(maskb[:], maskb_d[:])
    flagv = wpool.tile([128, 1], F32, tag="flagv", name="flagv")
    flag2 = wpool.tile([128, 1], F32, tag="flag2", name="flag2")
    nc.sync.dma_start(flagv[:], flagv_d[:])
    nc.sync.dma_start(flag2[:], flag2_d[:])
    ffb = [wpool.tile([128, 1], F32, tag=f"ffb{i}", name=f"ffb{i}") for i in range(DT)]
    for i in range(DT):
        nc.sync.dma_start(ffb[i][:], ffb_d[i * 128:(i + 1) * 128, :])
    w_inT = [wpool.tile([128, 2 * E], FP16, tag=f"w_inT{i}", name=f"w_inT{i}") for i in range(DT)]
    for i in range(DT):
        nc.sync.dma_start(w_inT[i][:], w_inT_d[i * 128:(i + 1) * 128, :])
    cw = [wpool.tile([128, K], F32, tag=f"cw{i}", name=f"cw{i}") for i in range(ET)]
    cb = [wpool.tile([128, 1], F32, tag=f"cb{i}", name=f"cb{i}") for i in range(ET)]
    dtb = [wpool.tile([128, 1], F32, tag=f"dtb{i}", name=f"dtb{i}") for i in range(ET)]
    dp = [wpool.tile([128, 1], F32, tag=f"dp{i}", name=f"dp{i}") for i in range(ET)]
    aneg = [wpool.tile([128, N], F32, tag=f"aneg{i}", name=f"aneg{i}") for i in range(ET)]
    w_xT = [wpool.tile([128, R + 2 * N], FP16, tag=f"w_xT{i}", name=f"w_xT{i}") for i in range(ET)]
    w_outT = [wpool.tile([128, D], FP16, tag=f"w_outT{i}", name=f"w_outT{i}") for i in range(ET)]
    for i in range(ET):
        sl = slice(i * 128, (i + 1) * 128)
        nc.sync.dma_start(cw[i][:], cw_d[sl, :])
        nc.sync.dma_start(cb[i][:], cb_d[sl, :])
        nc.sync.dma_start(dtb[i][:], dtb_d[sl, :])
        nc.sync.dma_start(dp[i][:], dp_d[sl, :])
        nc.sync.dma_start(aneg[i][:], aneg_d[sl, :])
        nc.sync.dma_start(w_xT[i][:], w_xT_d[sl, :])
        nc.sync.dma_start(w_outT[i][:], w_outT_d[sl, :])
    w_dtT = wpool.tile([R, E], FP16, tag="w_dtT", name="w_dtT")
    nc.sync.dma_start(w_dtT[:], w_dtT_d[:])

    ident = wpool.tile([128, 128], F32, tag="ident", name="ident")
    masks.make_identity(nc, ident[:])
    identb = wpool.tile([128, 128], FP16, tag="identb", name="identb")
    nc.vector.tensor_copy(identb[:], ident[:])
    # flag-scaled identity: adds resid into the out_proj psum on self cores only
    flagident = wpool.tile([128, 128], FP16, tag="flagident", name="flagident")
    nc.vector.tensor_scalar_mul(flagident[:], ident[:], flagv[:])
    diag_dp = [wpool.tile([128, 128], FP16, tag=f"ddp{e}", name=f"ddp{e}") for e in range(ET)]
    for e in range(ET):
        nc.vector.tensor_scalar_mul(diag_dp[e][:], ident[:], dp[e][:])
    diag_cw = [[stage2.tile([128, 128], FP16, tag=f"dcw{e}_{k}", name=f"dcw{e}_{k}") for k in range(K)]
               for e in range(ET)]
    for e in range(ET):
        for k in range(K):
            nc.vector.tensor_scalar_mul(diag_cw[e][k][:], ident[:], cw[e][:, k:k + 1])

    # ---------------- conv_ff -> convout (fp16) + resid (fp16) ----------------
    convout = [wff.tile([128, L], FP16, tag=f"convout{i}", name=f"convout{i}") for i in range(DT)]
    resid_dram = dramp.tile([D, L], FP16, tag="residd", name="residd")
    for do in range(DT):
        for t in range(TC):
            ps = mmwp.tile([128, 512], F32, tag="mmw", name="psmmw")
            nmm = K * DT
            i = 0
            for k in range(K):
                for di in range(DT):
                    nc.tensor.matmul(
                        ps[:],
                        ffw[di][:, k * D + do * 128: k * D + (do + 1) * 128],
                        xin16[di][:, t * 512 + k: t * 512 + k + 512],
                        start=(i == 0), stop=(i == nmm - 1),
                    )
                    i += 1
            nc.scalar.activation(convout[do][:, t * 512:(t + 1) * 512], ps[:], AF.Relu, bias=ffb[do][:])
            resid_s = chunk.tile([128, 512], FP16, tag="resid_s", name="resid_s")
            nc.vector.tensor_tensor(
                resid_s[:],
                convout[do][:, t * 512:(t + 1) * 512],
                xin16[do][:, 3 + t * 512: 3 + (t + 1) * 512], ADD,
            )
            nc.sync.dma_start(
                resid_dram[do * 128:(do + 1) * 128, t * 512:(t + 1) * 512], resid_s[:])

    # ---------------- instance norm -> mamba_in (fp16) ----------------
    # norm input: self cores = convout, cross cores = xin  (flagv selects)
    mamba_in = [stage3.tile([128, L], FP16, tag=f"mambain{i}", name=f"mambain{i}") for i in range(DT)]
    for i in range(DT):
        nin = chunk.tile([128, L], FP16, tag="nin2048", name="nin")
        ssum = wpool.tile([128, 1], F32, tag=f"ssum{i}", name=f"ssum{i}")
        ssq = wpool.tile([128, 1], F32, tag=f"ssq{i}", name=f"ssq{i}")
        nc.vector.tensor_scalar_mul(nin[:], xin16[i][:, 3:L + 3], flag2[:])
        nc.vector.scalar_tensor_tensor(nin[:], convout[i][:], flagv[:], nin[:], MULT, ADD,
                                       accum_out=ssum[:])
        trash = chunk.tile([128, L], FP16, tag="nin2048", name="trash")
        nc.scalar.activation(trash[:], nin[:], AF.Square, accum_out=ssq[:])
        mean = wpool.tile([128, 1], F32, tag=f"mean{i}", name=f"mean{i}")
        var = wpool.tile([128, 1], F32, tag=f"var{i}", name=f"var{i}")
        nc.vector.tensor_scalar_mul(mean[:], ssum[:], 1.0 / L)
        nc.vector.tensor_scalar_mul(var[:], ssq[:], 1.0 / L)
        msq = wpool.tile([128, 1], F32, tag=f"msq{i}", name=f"msq{i}")
        nc.vector.tensor_tensor(msq[:], mean[:], mean[:], MULT)
        nc.vector.scalar_tensor_tensor(var[:], msq[:], -1.0, var[:], MULT, ADD)
        nc.vector.tensor_scalar_add(var[:], var[:], EPS)
        inv = wpool.tile([128, 1], F32, tag=f"inv{i}", name=f"inv{i}")
        nc.vector.reciprocal(inv[:], var[:])
        nc.scalar.sqrt(inv[:], inv[:])
        muinv = wpool.tile([128, 1], F32, tag=f"muinv{i}", name=f"muinv{i}")
        nc.vector.tensor_tensor(muinv[:], mean[:], inv[:], MULT)
        nc.vector.tensor_scalar(mamba_in[i][:], nin[:], inv[:], muinv[:], MULT, SUB)
    wff.release()

    # ---------------- in_proj (xp half) ----------------
    xp = [stage2.tile([128, L + 6], FP16, tag=f"xp{e}", name=f"xp{e}") for e in range(ET)]
    for e in range(ET):
        nc.gpsimd.memset(xp[e][:, 0:6], 0.0)
        for t in range(TC):
            ps = mmwp.tile([128, 512], F32, tag="mmw", name="psmmw")
            for di in range(DT):
                nc.tensor.matmul(
                    ps[:], w_inT[di][:, e * 128:(e + 1) * 128],
                    mamba_in[di][:, t * 512:(t + 1) * 512],
                    start=(di == 0), stop=(di == DT - 1),
                )
            nc.vector.tensor_copy(xp[e][:, 6 + t * 512: 6 + (t + 1) * 512], ps[:])

    # ---------------- gate half: zs = silu(in_proj_z), spilled to DRAM ----------------
    zs_dram = dramp.tile([E, L], FP16, tag="zsd", name="zsd")
    for e in range(ET):
        for t in range(TC):
            ps = psmall.tile([128, 512], F32, tag="mm", name="psmm")
            for di in range(DT):
                nc.tensor.matmul(
                    ps[:], w_inT[di][:, E + e * 128: E + (e + 1) * 128],
                    mamba_in[di][:, t * 512:(t + 1) * 512],
                    start=(di == 0), stop=(di == DT - 1),
                )
            zs_s = chunk.tile([128, 512], FP16, tag="zs_s", name="zs_s")
            nc.scalar.activation(zs_s[:], ps[:], AF.Silu)
            nc.sync.dma_start(
                zs_dram[e * 128:(e + 1) * 128, t * 512:(t + 1) * 512], zs_s[:])
    stage3.release()

    # ---------------- depthwise causal conv + silu -> u ----------------
    u = [big.tile([128, L], FP16, tag=f"u{e}", name=f"u{e}") for e in range(ET)]
    for e in range(ET):
        for t in range(TC):
            ps = mmwp.tile([128, 512], F32, tag="mmw", name="psmmw")
            for k in range(K):
                nc.tensor.matmul(
                    ps[:], diag_cw[e][k][:],
                    xp[e][:, t * 512 + k: t * 512 + k + 512],
                    start=(k == 0), stop=(k == K - 1),
                )
            nc.scalar.activation(u[e][:, t * 512:(t + 1) * 512], ps[:], AF.Silu, bias=cb[e][:])
    stage2.release()

    # ---------------- x_proj -> xdblR (fp16) + B/C rows to DRAM ----------------
    xdblR = big.tile([R, L], FP16, tag="xdblR", name="xdblR")
    bcsrc = big.tile([2 * N, L], FP16, tag="bcsrc", name="bcsrc")
    for t in range(TC):
        ps = mmwp.tile([R + 2 * N, 512], F32, tag="mmw", name="psmmx")
        for e in range(ET):
            nc.tensor.matmul(
                ps[:], w_xT[e][:], u[e][:, t * 512:(t + 1) * 512],
                start=(e == 0), stop=(e == ET - 1),
            )
        nc.scalar.copy(bcsrc[:, t * 512:(t + 1) * 512], ps[0:2 * N, :])
        nc.scalar.copy(xdblR[:, t * 512:(t + 1) * 512], ps[2 * N:2 * N + R, :])
    bc_dram = dramp.tile([2 * N, L], FP16, tag="bcdram", name="bcdram")
    for t in range(TC):
        nc.sync.dma_start(bc_dram[:, t * 512:(t + 1) * 512],
                          bcsrc[:, t * 512:(t + 1) * 512])

    bcp = pool("bcast", 4)
    scanp = pool("scan", 2)

    # ---------------- dt_proj -> dt = -softplus(...) via ln(sigmoid(-x)) ----------------
    # (no Softplus table on gen3; sigmoid/ln batched over e-tile PAIRS to
    # halve the Act table reloads)
    dt = [big.tile([128, L], FP16, tag=f"dt{e}", name=f"dt{e}") for e in range(ET)]
    w = [big.tile([128, L], FP16, tag=f"w{e}", name=f"w{e}") for e in range(ET)]
    # pair 0 (e0,e1): full dt chain now — pass 0 needs it to start scanning
    lntmps = {}
    for e in (0, 1):
        lntmp = chunk.tile([128, L], F32, tag="lntmp", name=f"lntmp{e}")
        lntmps[e] = lntmp
        for t in range(TC):
            ps = psmall.tile([128, 512], F32, tag="mm", name="psmm")
            nc.tensor.matmul(
                ps[:], w_dtT[:, e * 128:(e + 1) * 128],
                xdblR[:, t * 512:(t + 1) * 512], start=True, stop=True,
            )
            nc.scalar.activation(lntmp[:, t * 512:(t + 1) * 512], ps[:], AF.Sigmoid,
                                 bias=dtb[e][:], scale=-1.0)
    for e in (0, 1):
        for t in range(TC):
            nc.scalar.activation(dt[e][:, t * 512:(t + 1) * 512],
                                 lntmps[e][:, t * 512:(t + 1) * 512], AF.Ln)
        nc.vector.tensor_tensor(w[e][:], dt[e][:], u[e][:], MULT)
    # pair 1 (e2,e3): run the matmuls now (psmall still alive) but stage the
    # raw projection into the unused w tiles via the idle front DVE; the
    # sigmoid/ln (Act) and w-muls are deferred into the scan phase's Act
    # slack so the first quad's Exps don't wait behind them
    for e in (2, 3):
        for t in range(TC):
            ps = psmall.tile([128, 512], F32, tag="mm", name="psmm")
            nc.tensor.matmul(
                ps[:], w_dtT[:, e * 128:(e + 1) * 128],
                xdblR[:, t * 512:(t + 1) * 512], start=True, stop=True,
            )
            nc.vector.tensor_copy(w[e][:, t * 512:(t + 1) * 512], ps[:])

    def emit_dt_pair1():
        lt = {}
        for e in (2, 3):
            lntmp = chunk.tile([128, L], F32, tag="lntmp", name=f"lntmp{e}")
            lt[e] = lntmp
            for t in range(TC):
                nc.scalar.activation(lntmp[:, t * 512:(t + 1) * 512],
                                     w[e][:, t * 512:(t + 1) * 512], AF.Sigmoid,
                                     bias=dtb[e][:], scale=-1.0)
        for e in (2, 3):
            for t in range(TC):
                nc.scalar.activation(dt[e][:, t * 512:(t + 1) * 512],
                                     lt[e][:, t * 512:(t + 1) * 512], AF.Ln)
            nc.vector.tensor_tensor(w[e][:], dt[e][:], u[e][:], MULT)

    # ---------------- selective scan ----------------
    # Concatenated scans: the tensor_tensor_scan instruction has a ~4us fixed
    # cost, so pack NCAT n-states into one long scan per (e, quad). State
    # resets at unit boundaries are free: h[0] = dA[0]*h[-1] + z[0] and the
    # boundary dA column is pinned to zero (Exp writes skip it; a per-set
    # memset clears it), so each unit starts from h=0 exactly.
    # n-outer over e-pairs: b/c broadcast once per (pass, n) instead of per
    # (e, n); PSUM holds two full-L f32 y-accumulators (8 banks).
    NCAT = 4
    LCAT = NCAT * L
    mmwp.release()
    psmall.release()
    pbig = pool("psum_y", 1, space="PSUM")       # 2x [128,2048] y accumulators
    yg = [None] * ET
    for pair in range(ET // 2):
        es = [2 * pair, 2 * pair + 1]
        py = {}
        for e in es:
            py[e] = pbig.tile([128, L], F32, tag=f"py{e - 2 * pair}", name=f"py{e}")
            for t in range(TC):
                nc.tensor.matmul(
                    py[e][:, t * 512:(t + 1) * 512], diag_dp[e][:],
                    u[e][:, t * 512:(t + 1) * 512],
                    start=True, stop=False,
                )
        for q in range(N // NCAT):
            ns = list(range(q * NCAT, (q + 1) * NCAT))
            bcs = []
            for n in ns:
                b_bc = bcp.tile([128, L], FP16, tag="bbc", name="bbc")
                c_bc = bcp.tile([128, L], FP16, tag="cbc", name="cbc")
                nc.sync.dma_start(b_bc[:], bc_dram[n, :].partition_broadcast(128))
                nc.sync.dma_start(c_bc[:], bc_dram[N + n, :].partition_broadcast(128))
                bcs.append((b_bc, c_bc))
            for e in es:
                dA = scanp.tile([128, LCAT], FP16, tag="dA", name="dA")
                z = scanp.tile([128, LCAT], FP16, tag="z", name="z")
                for k, n in enumerate(ns):
                    if k > 0:
                        nc.vector.memset(dA[:, k * L:k * L + 1], 0.0)
                    off = k * L + (1 if k > 0 else 0)
                    nc.scalar.activation(dA[:, off:(k + 1) * L],
                                         dt[e][:, off - k * L:L], AF.Exp,
                                         scale=aneg[e][:, n:n + 1])
                    # z slices all on DVE (Pool SBUF-port contention test)
                    nc.vector.tensor_tensor(z[:, k * L:(k + 1) * L], w[e][:],
                                            bcs[k][0][:], MULT)
                # in-place scan: h overwrites z (same-position write-behind)
                nc.vector.tensor_tensor_scan(z[:], dA[:], z[:], 0.0, MULT, ADD)
                # yp slices in-place over dA, then accumulate into py via PE
                for k, n in enumerate(ns):
                    nc.vector.tensor_tensor(dA[:, k * L:(k + 1) * L],
                                            z[:, k * L:(k + 1) * L],
                                            bcs[k][1][:], MULT)
                    for t in range(TC):
                        nc.tensor.matmul(
                            py[e][:, t * 512:(t + 1) * 512], identb[:],
                            dA[:, k * L + t * 512: k * L + (t + 1) * 512],
                            start=False, stop=(q == N // NCAT - 1 and k == NCAT - 1),
                        )
                # gating emitted per-e right after its accumulator closes, so
                # e0's gating overlaps e1's scan work in the final quad
                if q == N // NCAT - 1:
                    yge = big.tile([128, L], FP16, tag=f"u{e}", name=f"yg{e}")
                    yg[e] = yge
                    for t in range(TC):
                        zs_l = chunk.tile([128, 512], FP16, tag="zs_s", name="zs_l")
                        nc.sync.dma_start(
                            zs_l[:], zs_dram[e * 128:(e + 1) * 128, t * 512:(t + 1) * 512])
                        py_s = chunk.tile([128, 512], FP16, tag="py_s", name="py_s")
                        nc.scalar.copy(py_s[:], py[e][:, t * 512:(t + 1) * 512])
                        nc.vector.tensor_tensor(
                            yge[:, t * 512:(t + 1) * 512],
                            py_s[:],
                            zs_l[:], MULT,
                        )
            if pair == 0 and q == 0:
                emit_dt_pair1()
    pbig.release()
    psout = pool("psum_out", 2, space="PSUM")

    # ---------------- out_proj + resid-add (via matmul) + mask + store ----------------
    for do in range(DT):
        pss = [psout.tile([128, 512], F32, tag=f"op{t}", bufs=1, name=f"op{do}_{t}")
               for t in range(TC)]
        for e in range(ET):
            for t in range(TC):
                nc.tensor.matmul(
                    pss[t][:], w_outT[e][:, do * 128:(do + 1) * 128],
                    yg[e][:, t * 512:(t + 1) * 512],
                    start=(e == 0), stop=False,
                )
        for t in range(TC):
            resid_l = chunk.tile([128, 512], FP16, tag="resid_s", name="resid_l")
            nc.sync.dma_start(
                resid_l[:], resid_dram[do * 128:(do + 1) * 128, t * 512:(t + 1) * 512])
            nc.tensor.matmul(
                pss[t][:], flagident[:],
                resid_l[:],
                start=False, stop=True,
            )
            ofin = chunk.tile([128, 512], F32, tag="ofin", name="ofin")
            nc.vector.tensor_tensor(ofin[:], pss[t][:], maskb[:, t * 512:(t + 1) * 512], MULT)
            nc.sync.dma_start(out_d[do * 128:(do + 1) * 128, t * 512:(t + 1) * 512], ofin[:])

    for p in reversed(_pools):
        if not p._released:
            p.release()


_NC_CACHE = {}


def _get_nc():
    if "nc" not in _NC_CACHE:
        _NC_CACHE["nc"] = build_nc()
    return _NC_CACHE["nc"]


def _core_inputs(blk, b, inputs):
    pfx = "s_" if blk == 0 else "c_"
    xin = inputs["x"][b] if blk == 0 else inputs["encoder_states"][b]
    f = 1.0 if blk == 0 else 0.0
    g = lambda k: np.asarray(inputs[pfx + k])
    aneg = np.exp(g("A_log"))  # = -A; dt tile holds -softplus so dA = exp(aneg*dt)
    return {
        "xin": np.ascontiguousarray(xin, np.float32),
        "maskb": np.ascontiguousarray(
            np.broadcast_to(inputs["padding_mask"][b][None, :], (128, L))).astype(np.float16),
        "flagv": np.full((128, 1), f, np.float32),
        "flag2": np.full((128, 1), 1.0 - f, np.float32),
        "ffw": np.ascontiguousarray(np.asarray(inputs["ff_w"]).transpose(2, 1, 0)).astype(np.float16),
        "ffb": np.asarray(inputs["ff_b"]).reshape(D, 1).astype(np.float32),
        "w_inT": np.ascontiguousarray(g("in_proj_w").T).astype(np.float16),
        "cw": np.ascontiguousarray(g("conv_w").reshape(E, K), np.float32),
        "cb": g("conv_b").reshape(E, 1).astype(np.float32),
        "w_xT": np.ascontiguousarray(
            g("x_proj_w").T[:, list(range(R, R + 2 * N)) + list(range(R))]
        ).astype(np.float16),
        "w_dtT": np.ascontiguousarray(g("dt_proj_w").T).astype(np.float16),
        "dtb": (-g("dt_proj_b")).reshape(E, 1).astype(np.float32),
        "aneg": np.ascontiguousarray(aneg, np.float32),
        "dp": (-g("D")).reshape(E, 1).astype(np.float32),
        "w_outT": np.ascontiguousarray(-g("out_proj_w").T).astype(np.float16),
    }


def kernel(**inputs):
    nc = _get_nc()
    in_maps = []
    for b in range(B):
        in_maps.append(_core_inputs(0, b, inputs))  # core 2b: self block
        in_maps.append(_core_inputs(1, b, inputs))  # core 2b+1: cross block
    res = run_bass_kernel_spmd(nc, in_maps, core_ids=list(range(8)))
    out = np.empty((B, D, L), np.float32)
    for b in range(B):
        out[b] = res.results[2 * b]["out"] + res.results[2 * b + 1]["out"]
    return out

